# revision 13
# baseline (speedup 1.0000x reference)
"""Trainium2 Bass kernel for the scatter_memory nn.Module problem.

Math (reference):
  read:  score = text @ cache.T; sm = softmax(score, axis=1); fine = sm @ cache
  ext:   tf = ALPHA * ([text, fine] @ W_ext.T) + text
  write: bf = l2norm(image); s = bf @ cache.T; sq = softmax(s, axis=0)
         assign = argmax(s, 1); w = sq[i, a_i] / colmax(sq)[a_i] = exp(s[i,a_i] - colmax(s)[a_i])
         slot_sum = segment_sum(w * bf, assign); counts = segment_sum(1, assign)
         new_cache = l2norm(where(counts>0, 0.8*cache + 0.2*slot_sum, cache))
  loss = mean |l2norm(tf) - text|

Sharding: data-parallel over tokens (C=32768 -> 4096/core on 8 cores).
Per-core outputs: tf shard, unnormalized slot sums u = sum_{i:a_i=j} exp(s_ij)*bf_i,
per-partition indicator sums / running col-max / loss partials. Host combines
(tiny [25,1024] work): slot_sum = exp(-M) * sum_c u_c, counts, new_cache, loss.

Device algebra:
  fine @ W2.T = sm @ (cache @ W2.T) = sm @ G  (G precomputed host-side, alpha-folded)
  tf = text @ (a*W1.T) + sm @ (a*G0) + text
"""

import numpy as np

import concourse.bass as bass
import concourse.bacc as bacc
import concourse.mybir as mybir
from concourse import tile
from concourse import bass_utils

F32 = mybir.dt.float32
F32R = mybir.dt.float32r  # fast fp32 matmul mode (rounded fp32)
AX = mybir.AxisListType
ALU = mybir.AluOpType
ACTF = mybir.ActivationFunctionType

C_FULL = 32768
D = 1024
M_SLOTS = 25
N_CORES = 8
CL = C_FULL // N_CORES          # 4096 tokens per core
GROUP = 512                     # tokens per pipeline group
N_GROUPS = CL // GROUP          # 8
SUBS = GROUP // 128             # 4 subtiles of 128 tokens per group
ALPHA = 0.2
MOMENTUM = 0.8




def build_nc():
    nc = bacc.Bacc("TRN2", target_bir_lowering=False, debug=False,
                   enable_asserts=False, num_devices=N_CORES)

    # ---- kernel I/O (per-core shard) ----
    tT = nc.dram_tensor("tT", [D, CL], F32R, kind="ExternalInput").ap()      # text.T shard
    t_r = nc.dram_tensor("t_r", [CL, D], F32, kind="ExternalInput").ap()    # text shard
    iT = nc.dram_tensor("iT", [D, CL], F32R, kind="ExternalInput").ap()      # image.T shard
    im_r = nc.dram_tensor("im_r", [CL, D], F32R, kind="ExternalInput").ap()  # image shard
    w1t = nc.dram_tensor("w1t", [D, D], F32R, kind="ExternalInput").ap()     # alpha*W1.T
    g_in = nc.dram_tensor("g_in", [M_SLOTS, D], F32R, kind="ExternalInput").ap()  # alpha*(cache@W2.T)
    ct_in = nc.dram_tensor("ct_in", [D, M_SLOTS], F32R, kind="ExternalInput").ap()  # cache.T
    ident_in = nc.dram_tensor("ident_in", [128, 128], F32, kind="ExternalInput").ap()
    ones_in = nc.dram_tensor("ones_in", [128, 32], F32R, kind="ExternalInput").ap()
    iota_in = nc.dram_tensor("iota_in", [128, M_SLOTS], F32, kind="ExternalInput").ap()
    rin_in = nc.dram_tensor("rin_in", [CL, 1], F32, kind="ExternalInput").ap()  # 1/||image||

    tf_out = nc.dram_tensor("tf_out", [CL, D], F32, kind="ExternalOutput").ap()
    u_out = nc.dram_tensor("u_out", [M_SLOTS, D], F32, kind="ExternalOutput").ap()
    # aux: cols 0:25 indicator sums, 25:50 running col-max of s, 50 loss partial
    aux_out = nc.dram_tensor("aux_out", [128, 51], F32, kind="ExternalOutput").ap()
    # aux2: per token (p, tile t): [3t]=top1 of s, [3t+1]=top2, [3t+2]=argmax idx
    aux2_out = nc.dram_tensor("aux2_out", [128, 3 * CL // 128], F32, kind="ExternalOutput").ap()

    with tile.TileContext(nc) as tc, nc.allow_low_precision(reason="fp32r matmul operands are rounded fp32"):
        with (
            tc.tile_pool(name="const", bufs=1) as constp,
            tc.tile_pool(name="wpool", bufs=1) as wpool,
            tc.tile_pool(name="bigin", bufs=2) as bigin,
            tc.tile_pool(name="rows", bufs=2) as rows,
            tc.tile_pool(name="work", bufs=2) as work,
            tc.tile_pool(name="small", bufs=2) as small,
            tc.tile_pool(name="stat", bufs=6) as stat,
            tc.tile_pool(name="acc", bufs=1) as accp,
            tc.tile_pool(name="ps_small", bufs=3, space="PSUM") as ps_small,
            tc.tile_pool(name="ps_big", bufs=2, space="PSUM") as ps_big,
            tc.tile_pool(name="ps_u", bufs=1, space="PSUM") as ps_u,
        ):
            # ---- persistent SBUF ----
            w1t_sb = wpool.tile([128, 8, D], F32R)     # 32KB/part
            for c in range(8):
                nc.sync.dma_start(w1t_sb[:, c, :], w1t[c * 128:(c + 1) * 128, :])
            ct_sb = constp.tile([128, 8, M_SLOTS], F32R)
            for c in range(8):
                nc.sync.dma_start(ct_sb[:, c, :], ct_in[c * 128:(c + 1) * 128, :])
            g_sb = constp.tile([M_SLOTS, D], F32R)
            nc.sync.dma_start(g_sb[:, :], g_in[:, :])
            ident = constp.tile([128, 128], F32)
            nc.sync.dma_start(ident[:, :], ident_in[:, :])
            ones_sb = constp.tile([128, 32], F32R)
            nc.sync.dma_start(ones_sb[:, :], ones_in[:, :])
            iota_sb = constp.tile([128, M_SLOTS], F32)
            nc.sync.dma_start(iota_sb[:, :], iota_in[:, :])

            # ---- accumulators ----
            u_psum = ps_u.tile([M_SLOTS, D], F32)          # 2 banks, lives all loop
            ind_acc = accp.tile([128, M_SLOTS], F32)
            cmax_acc = accp.tile([128, M_SLOTS], F32)
            loss_cols = accp.tile([128, CL // 128], F32)
            aux2_acc = accp.tile([128, 3 * CL // 128], F32)
            nc.vector.memset(ind_acc[:, :], 0.0)
            nc.vector.memset(cmax_acc[:, :], -3.0e38)

            for grp in range(N_GROUPS):
                g0 = grp * GROUP

                # ---------- loads ----------
                ttg = bigin.tile([128, 8, GROUP], F32R, tag="ttg")
                for c in range(8):
                    nc.sync.dma_start(ttg[:, c, :], tT[c * 128:(c + 1) * 128, g0:g0 + GROUP])
                itg = bigin.tile([128, 8, GROUP], F32R, tag="itg")
                for c in range(8):
                    nc.sync.dma_start(itg[:, c, :], iT[c * 128:(c + 1) * 128, g0:g0 + GROUP])

                # ---------- read path: softmax over slots ----------
                scT = ps_small.tile([M_SLOTS, GROUP], F32, tag="pss")
                for c in range(8):
                    nc.tensor.matmul(scT[:, :], (ct_sb[:, c, :]), (ttg[:, c, :]),
                                     start=(c == 0), stop=(c == 7))
                escT = small.tile([M_SLOTS, GROUP], F32R, tag="escT")
                nc.scalar.activation(escT[:, :], scT[:, :], ACTF.Exp)
                s_sum = ps_small.tile([1, GROUP], F32, tag="pss")
                nc.tensor.matmul(s_sum[:, :], (ones_sb[:M_SLOTS, :1]), (escT[:, :]),
                                 start=True, stop=True)
                r_sum = small.tile([1, GROUP], F32R, tag="rsum")
                nc.vector.reciprocal(r_sum[:, :], s_sum[:, :])
                bcast = ps_small.tile([M_SLOTS, GROUP], F32, tag="pss")
                nc.tensor.matmul(bcast[:, :], (ones_sb[:1, :M_SLOTS]), (r_sum[:, :]),
                                 start=True, stop=True)
                smT = small.tile([M_SLOTS, GROUP], F32R, tag="smT")
                nc.vector.tensor_tensor(smT[:, :], escT[:, :], bcast[:, :], ALU.mult)

                # ---------- write path: raw scores ----------
                srT = ps_small.tile([M_SLOTS, GROUP], F32, tag="pss")
                for c in range(8):
                    # full fp32 matmul: argmax over s must match the fp32 reference
                    nc.tensor.matmul(srT[:, :], ct_sb[:, c, :].bitcast(F32),
                                     itg[:, c, :].bitcast(F32),
                                     start=(c == 0), stop=(c == 7))
                sr_sb = small.tile([M_SLOTS, GROUP], F32, tag="sr_sb")
                nc.scalar.copy(sr_sb[:, :], srT[:, :])

                for s in range(SUBS):
                    tok0 = g0 + s * 128
                    tsl = slice(s * 128, (s + 1) * 128)
                    tile_idx = tok0 // 128

                    # ---- normalized image: rin precomputed on host ----
                    imt = rows.tile([128, D], F32R, tag="imt")
                    nc.sync.dma_start(imt[:, :], im_r[tok0:tok0 + 128, :])
                    rin = stat.tile([128, 1], F32, tag="rin")
                    nc.sync.dma_start(rin[:, :], rin_in[tok0:tok0 + 128, :])

                    # ---- s tile token-major ----
                    ps_t = ps_small.tile([128, M_SLOTS], F32, tag="pss")
                    nc.tensor.transpose(ps_t[:, :], sr_sb[:, tsl], ident[:M_SLOTS, :M_SLOTS])
                    s_tok = small.tile([128, M_SLOTS], F32, tag="s_tok")
                    nc.vector.tensor_scalar_mul(s_tok[:, :], ps_t[:, :], rin[:, :])

                    # ---- top-1 mask & weights ----
                    rmax = aux2_acc[:, 3 * tile_idx:3 * tile_idx + 1]
                    nc.vector.tensor_reduce(rmax, s_tok[:, :], AX.X, ALU.max)
                    e_tok = small.tile([128, M_SLOTS], F32, tag="e_tok")
                    nc.scalar.activation(e_tok[:, :], s_tok[:, :], ACTF.Exp)
                    ind = small.tile([128, M_SLOTS], F32, tag="ind")
                    nc.vector.tensor_scalar(ind[:, :], s_tok[:, :], rmax, None,
                                            op0=ALU.is_ge)
                    # top-2 value and argmax index (for host-side near-tie fixup)
                    msk = small.tile([128, M_SLOTS], F32, tag="msk")
                    nc.vector.scalar_tensor_tensor(msk[:, :], ind[:, :], -1.0e30,
                                                   s_tok[:, :], ALU.mult, ALU.add)
                    nc.vector.tensor_reduce(aux2_acc[:, 3 * tile_idx + 1:3 * tile_idx + 2],
                                            msk[:, :], AX.X, ALU.max)
                    nc.vector.tensor_tensor(msk[:, :], ind[:, :], iota_sb[:, :], ALU.mult)
                    nc.vector.tensor_reduce(aux2_acc[:, 3 * tile_idx + 2:3 * tile_idx + 3],
                                            msk[:, :], AX.X, ALU.max)
                    er = small.tile([128, M_SLOTS], F32, tag="er")
                    nc.vector.tensor_scalar_mul(er[:, :], e_tok[:, :], rin[:, :])
                    wm = small.tile([128, M_SLOTS], F32R, tag="wm")
                    nc.vector.tensor_tensor(wm[:, :], er[:, :], ind[:, :], ALU.mult)
                    nc.vector.tensor_tensor(cmax_acc[:, :], cmax_acc[:, :], s_tok[:, :], ALU.max)
                    nc.vector.tensor_tensor(ind_acc[:, :], ind_acc[:, :], ind[:, :], ALU.add)

                    # ---- slot sums (segment-sum as matmul) ----
                    first = (tile_idx == 0)
                    last = (tile_idx == CL // 128 - 1)
                    for h in range(2):
                        nc.tensor.matmul(u_psum[:, h * 512:(h + 1) * 512], (wm[:, :]),
                                         (imt[:, h * 512:(h + 1) * 512]),
                                         start=first, stop=last)

                    # ---- extractor ----
                    t_sb = rows.tile([128, D], F32, tag="t_sb")
                    nc.sync.dma_start(t_sb[:, :], t_r[tok0:tok0 + 128, :])
                    tf_sb = rows.tile([128, D], F32, tag="tf_sb")
                    for h in range(2):
                        hsl = slice(h * 512, (h + 1) * 512)
                        tfp = ps_big.tile([128, 512], F32, tag="tfp")
                        for c in range(8):
                            nc.tensor.matmul(tfp[:, :], (ttg[:, c, tsl]),
                                             (w1t_sb[:, c, hsl]),
                                             start=(c == 0), stop=False)
                        nc.tensor.matmul(tfp[:, :], (smT[:, tsl]), (g_sb[:, hsl]),
                                         start=False, stop=True)
                        nc.vector.tensor_tensor(tf_sb[:, hsl], tfp[:, :], t_sb[:, hsl], ALU.add)
                    nc.sync.dma_start(tf_out[tok0:tok0 + 128, :], tf_sb[:, :])

                    # ---- loss ----
                    ss2 = stat.tile([128, 1], F32, tag="ss2")
                    sq_scr = work.tile([128, D], F32, tag="scr")
                    nc.scalar.activation(sq_scr[:, :], tf_sb[:, :], ACTF.Square,
                                         accum_out=ss2[:, :])
                    ln2 = stat.tile([128, 1], F32, tag="ln2")
                    nc.scalar.activation(ln2[:, :], ss2[:, :], ACTF.Ln)
                    rn2 = stat.tile([128, 1], F32, tag="rn2")
                    nc.scalar.activation(rn2[:, :], ln2[:, :], ACTF.Exp, scale=-0.5)
                    diff = work.tile([128, D], F32, tag="diff")
                    nc.vector.scalar_tensor_tensor(diff[:, :], tf_sb[:, :], rn2[:, :],
                                                   t_sb[:, :], ALU.mult, ALU.subtract)
                    abs_scr = work.tile([128, D], F32, tag="scr")
                    nc.scalar.activation(abs_scr[:, :], diff[:, :], ACTF.Abs,
                                         accum_out=loss_cols[:, tile_idx:tile_idx + 1])

            # ---- finalize ----
            u_sb = wpool.tile([M_SLOTS, D], F32)
            nc.scalar.copy(u_sb[:, :], u_psum[:, :])
            nc.sync.dma_start(u_out[:, :], u_sb[:, :])
            aux_sb = accp.tile([128, 51], F32)
            nc.vector.tensor_copy(aux_sb[:, 0:M_SLOTS], ind_acc[:, :])
            nc.vector.tensor_copy(aux_sb[:, M_SLOTS:2 * M_SLOTS], cmax_acc[:, :])
            nc.vector.tensor_reduce(aux_sb[:, 50:51], loss_cols[:, :], AX.X, ALU.add)
            nc.sync.dma_start(aux_out[:, :], aux_sb[:, :])
            nc.sync.dma_start(aux2_out[:, :], aux2_acc[:, :])

    nc.compile()
    return nc


def _l2norm_np(x, axis=-1, eps=1e-12):
    n = np.linalg.norm(x, axis=axis, keepdims=True)
    return x / np.maximum(n, eps)


def make_in_maps(text_token, image_token, cache, W_ext):
    text_token = np.asarray(text_token, dtype=np.float32)
    image_token = np.asarray(image_token, dtype=np.float32)
    cache = np.asarray(cache, dtype=np.float32)
    W_ext = np.asarray(W_ext, dtype=np.float32)

    tT = np.ascontiguousarray(text_token.T)
    iT = np.ascontiguousarray(image_token.T)
    w1t = np.ascontiguousarray((ALPHA * W_ext[:, :D]).T)
    g = np.ascontiguousarray(ALPHA * (cache @ W_ext[:, D:].T))
    ct = np.ascontiguousarray(cache.T)
    ident = np.eye(128, dtype=np.float32)
    nrm = np.linalg.norm(image_token.astype(np.float64), axis=1, keepdims=True)
    rin_full = (1.0 / np.maximum(nrm, 1e-12)).astype(np.float32)

    in_maps = []
    for c in range(N_CORES):
        sl = slice(c * CL, (c + 1) * CL)
        in_maps.append({
            "tT": np.ascontiguousarray(tT[:, sl]),
            "t_r": np.ascontiguousarray(text_token[sl]),
            "iT": np.ascontiguousarray(iT[:, sl]),
            "im_r": np.ascontiguousarray(image_token[sl]),
            "w1t": w1t,
            "g_in": g,
            "ct_in": ct,
            "ident_in": ident,
            "ones_in": np.ones((128, 32), dtype=np.float32),
            "iota_in": np.tile(np.arange(M_SLOTS, dtype=np.float32), (128, 1)),
            "rin_in": rin_full[sl],
        })
    return in_maps


GAP_THR = 1.0e-3


def combine_outputs(results, cache, image_token):
    """results: list of 8 dicts with tf_out/u_out/aux_out/aux2_out. Host unshard.

    Near-tie argmax fixup: the device's s is fp32-accurate but summation order
    differs from the reference; for tokens whose top-2 gap < GAP_THR, recompute
    the slot scores in float64 and move the contribution if the device picked a
    different slot than the (essentially exact) fp64 argmax.
    """
    cache = np.asarray(cache, dtype=np.float32)
    tf = np.concatenate([r["tf_out"] for r in results], axis=0)

    u_sum = np.sum([r["u_out"] for r in results], axis=0).astype(np.float64)  # [25, 1024]
    ind_sum = np.sum([r["aux_out"][:, :M_SLOTS] for r in results], axis=(0, 1)).astype(np.float64)
    col_max = np.max([r["aux_out"][:, M_SLOTS:2 * M_SLOTS] for r in results], axis=(0, 1))
    loss = np.sum([r["aux_out"][:, 50] for r in results]) / (C_FULL * D)

    cache64 = cache.astype(np.float64)
    image_token = np.asarray(image_token)
    for c, r in enumerate(results):
        a2 = r["aux2_out"].astype(np.float64)          # [128, 3*ntiles]
        top1 = a2[:, 0::3]                              # [128, 32]
        top2 = a2[:, 1::3]
        idx = a2[:, 2::3]
        amb = np.argwhere(top1 - top2 < GAP_THR)
        for p, t in amb:
            gi = c * CL + t * 128 + p
            imr = image_token[gi].astype(np.float64)
            bf = imr / max(np.linalg.norm(imr), 1e-12)
            srow = bf @ cache64.T
            j_true = int(np.argmax(srow))
            j_dev = int(round(idx[p, t]))
            if j_dev != j_true:
                u_sum[j_dev] -= np.exp(srow[j_dev]) * bf
                u_sum[j_true] += np.exp(srow[j_true]) * bf
                ind_sum[j_dev] -= 1.0
                ind_sum[j_true] += 1.0

    slot_sum = u_sum * np.exp(-col_max)[:, None]
    counts = ind_sum
    updated = np.where(counts[:, None] > 0,
                       MOMENTUM * cache + (1.0 - MOMENTUM) * slot_sum,
                       cache)
    new_cache = _l2norm_np(updated).astype(np.float32)
    return tf.astype(np.float32), np.float32(loss), new_cache


_NC_CACHE = {}


def kernel(text_token, image_token, cache, W_ext):
    if "nc" not in _NC_CACHE:
        _NC_CACHE["nc"] = build_nc()
    nc = _NC_CACHE["nc"]
    in_maps = make_in_maps(text_token, image_token, cache, W_ext)
    res = bass_utils.run_bass_kernel_spmd(nc, in_maps, core_ids=list(range(N_CORES)))
    return combine_outputs(res.results, cache, image_token)


# revision 17
# speedup vs baseline: 1.0134x; 1.0134x over previous
"""Trainium2 Bass kernel for the scatter_memory nn.Module problem.

Math (reference):
  read:  score = text @ cache.T; sm = softmax(score, axis=1); fine = sm @ cache
  ext:   tf = ALPHA * ([text, fine] @ W_ext.T) + text
  write: bf = l2norm(image); s = bf @ cache.T; sq = softmax(s, axis=0)
         assign = argmax(s, 1); w = sq[i, a_i] / colmax(sq)[a_i] = exp(s[i,a_i] - colmax(s)[a_i])
         slot_sum = segment_sum(w * bf, assign); counts = segment_sum(1, assign)
         new_cache = l2norm(where(counts>0, 0.8*cache + 0.2*slot_sum, cache))
  loss = mean |l2norm(tf) - text|

Sharding: data-parallel over tokens (C=32768 -> 4096/core on 8 cores).
Per-core outputs: tf shard, unnormalized slot sums u = sum_{i:a_i=j} exp(s_ij)*bf_i,
per-partition indicator sums / running col-max / loss partials. Host combines
(tiny [25,1024] work): slot_sum = exp(-M) * sum_c u_c, counts, new_cache, loss.

Device algebra:
  fine @ W2.T = sm @ (cache @ W2.T) = sm @ G  (G precomputed host-side, alpha-folded)
  tf = text @ (a*W1.T) + sm @ (a*G0) + text
"""

import numpy as np

import concourse.bass as bass
import concourse.bacc as bacc
import concourse.mybir as mybir
from concourse import tile
from concourse import bass_utils

F32 = mybir.dt.float32
F32R = mybir.dt.float32r  # fast fp32 matmul mode (rounded fp32)
BF16 = mybir.dt.bfloat16
AX = mybir.AxisListType
ALU = mybir.AluOpType
ACTF = mybir.ActivationFunctionType

C_FULL = 32768
D = 1024
M_SLOTS = 25
N_CORES = 8
CL = C_FULL // N_CORES          # 4096 tokens per core
GROUP = 512                     # tokens per pipeline group
N_GROUPS = CL // GROUP          # 8
SUBS = GROUP // 128             # 4 subtiles of 128 tokens per group
ALPHA = 0.2
MOMENTUM = 0.8




def build_nc():
    nc = bacc.Bacc("TRN2", target_bir_lowering=False, debug=False,
                   enable_asserts=False, num_devices=N_CORES)

    # ---- kernel I/O (per-core shard) ----
    tT = nc.dram_tensor("tT", [D, CL], BF16, kind="ExternalInput").ap()      # text.T shard
    t_r = nc.dram_tensor("t_r", [CL, D], F32, kind="ExternalInput").ap()    # text shard
    iT = nc.dram_tensor("iT", [D, CL], F32R, kind="ExternalInput").ap()      # image.T shard
    im_r = nc.dram_tensor("im_r", [CL, D], F32R, kind="ExternalInput").ap()  # image shard
    w1t = nc.dram_tensor("w1t", [D, D], BF16, kind="ExternalInput").ap()     # alpha*W1.T
    g_in = nc.dram_tensor("g_in", [M_SLOTS, D], BF16, kind="ExternalInput").ap()  # alpha*(cache@W2.T)
    ct_in = nc.dram_tensor("ct_in", [D, M_SLOTS], F32R, kind="ExternalInput").ap()  # cache.T fp32
    ct_bf_in = nc.dram_tensor("ct_bf_in", [D, M_SLOTS], BF16, kind="ExternalInput").ap()  # cache.T
    ident_in = nc.dram_tensor("ident_in", [128, 128], F32, kind="ExternalInput").ap()
    ones_in = nc.dram_tensor("ones_in", [128, 32], F32R, kind="ExternalInput").ap()
    iota_in = nc.dram_tensor("iota_in", [128, M_SLOTS], F32, kind="ExternalInput").ap()
    rin_in = nc.dram_tensor("rin_in", [CL, 1], F32, kind="ExternalInput").ap()  # 1/||image||

    tf_out = nc.dram_tensor("tf_out", [CL, D], F32, kind="ExternalOutput").ap()
    u_out = nc.dram_tensor("u_out", [M_SLOTS, D], F32, kind="ExternalOutput").ap()
    # aux: cols 0:25 indicator sums, 25:50 running col-max of s, 50 loss partial
    aux_out = nc.dram_tensor("aux_out", [128, 51], F32, kind="ExternalOutput").ap()
    # aux2: per token (p, tile t): [3t]=top1 of s, [3t+1]=top2, [3t+2]=argmax idx
    aux2_out = nc.dram_tensor("aux2_out", [128, 3 * CL // 128], F32, kind="ExternalOutput").ap()

    with tile.TileContext(nc) as tc, nc.allow_low_precision(reason="fp32r matmul operands are rounded fp32"):
        with (
            tc.tile_pool(name="const", bufs=1) as constp,
            tc.tile_pool(name="wpool", bufs=1) as wpool,
            tc.tile_pool(name="bigin", bufs=2) as bigin,
            tc.tile_pool(name="rows", bufs=2) as rows,
            tc.tile_pool(name="work", bufs=2) as work,
            tc.tile_pool(name="small", bufs=2) as small,
            tc.tile_pool(name="stat", bufs=6) as stat,
            tc.tile_pool(name="acc", bufs=1) as accp,
            tc.tile_pool(name="ps_small", bufs=3, space="PSUM") as ps_small,
            tc.tile_pool(name="ps_big", bufs=3, space="PSUM") as ps_big,
            tc.tile_pool(name="ps_u", bufs=1, space="PSUM") as ps_u,
        ):
            # ---- persistent SBUF ----
            w1t_sb = wpool.tile([128, 8, D], BF16)     # 32KB/part
            for c in range(8):
                nc.sync.dma_start(w1t_sb[:, c, :], w1t[c * 128:(c + 1) * 128, :])
            ct_sb = constp.tile([128, 8, M_SLOTS], F32R)
            for c in range(8):
                nc.sync.dma_start(ct_sb[:, c, :], ct_in[c * 128:(c + 1) * 128, :])
            ct_bf = constp.tile([128, 8, M_SLOTS], BF16)
            for c in range(8):
                nc.sync.dma_start(ct_bf[:, c, :], ct_bf_in[c * 128:(c + 1) * 128, :])
            g_sb = constp.tile([M_SLOTS, D], BF16)
            nc.sync.dma_start(g_sb[:, :], g_in[:, :])
            ident = constp.tile([128, 128], F32)
            nc.sync.dma_start(ident[:, :], ident_in[:, :])
            ones_sb = constp.tile([128, 32], F32R)
            nc.sync.dma_start(ones_sb[:, :], ones_in[:, :])
            iota_sb = constp.tile([128, M_SLOTS], F32)
            nc.sync.dma_start(iota_sb[:, :], iota_in[:, :])

            # ---- accumulators ----
            u_psum = ps_u.tile([M_SLOTS, D], F32)          # 2 banks, lives all loop
            ind_acc = accp.tile([128, M_SLOTS], F32)
            cmax_acc = accp.tile([128, M_SLOTS], F32)
            loss_cols = accp.tile([128, CL // 128], F32)
            aux2_acc = accp.tile([128, 3 * CL // 128], F32)
            nc.vector.memset(ind_acc[:, :], 0.0)
            nc.vector.memset(cmax_acc[:, :], -3.0e38)

            for grp in range(N_GROUPS):
                g0 = grp * GROUP

                # ---------- loads ----------
                ttg = bigin.tile([128, 8, GROUP], BF16, tag="ttg")
                nc.sync.dma_start(ttg[:, :, :],
                                  tT.rearrange("(c p) t -> p c t", p=128)[:, :, g0:g0 + GROUP])
                itg = bigin.tile([128, 8, GROUP], F32R, tag="itg")
                nc.sync.dma_start(itg[:, :, :],
                                  iT.rearrange("(c p) t -> p c t", p=128)[:, :, g0:g0 + GROUP])

                # ---------- read path: softmax over slots ----------
                scT = ps_small.tile([M_SLOTS, GROUP], F32, tag="pss")
                for c in range(8):
                    nc.tensor.matmul(scT[:, :], ct_bf[:, c, :], ttg[:, c, :],
                                     start=(c == 0), stop=(c == 7))
                escT = small.tile([M_SLOTS, GROUP], F32R, tag="escT")
                nc.scalar.activation(escT[:, :], scT[:, :], ACTF.Exp)
                s_sum = ps_small.tile([1, GROUP], F32, tag="pss")
                nc.tensor.matmul(s_sum[:, :], (ones_sb[:M_SLOTS, :1]), (escT[:, :]),
                                 start=True, stop=True)
                ln_s = small.tile([1, GROUP], F32, tag="lns")
                nc.scalar.activation(ln_s[:, :], s_sum[:, :], ACTF.Ln)
                r_sum = small.tile([1, GROUP], F32R, tag="rsum")
                nc.scalar.activation(r_sum[:, :], ln_s[:, :], ACTF.Exp, scale=-1.0)
                bcast = ps_small.tile([M_SLOTS, GROUP], F32, tag="pss")
                nc.tensor.matmul(bcast[:, :], (ones_sb[:1, :M_SLOTS]), (r_sum[:, :]),
                                 start=True, stop=True)
                smT = small.tile([M_SLOTS, GROUP], BF16, tag="smT")
                nc.vector.tensor_tensor(smT[:, :], escT[:, :], bcast[:, :], ALU.mult)

                # ---------- write path: raw scores ----------
                srT = ps_small.tile([M_SLOTS, GROUP], F32, tag="pss")
                for c in range(8):
                    nc.tensor.matmul(srT[:, :], ct_sb[:, c, :], itg[:, c, :],
                                     start=(c == 0), stop=(c == 7))
                sr_sb = small.tile([M_SLOTS, GROUP], F32, tag="sr_sb")
                nc.scalar.copy(sr_sb[:, :], srT[:, :])

                for s in range(SUBS):
                    tok0 = g0 + s * 128
                    tsl = slice(s * 128, (s + 1) * 128)
                    tile_idx = tok0 // 128

                    # ---- normalized image: rin precomputed on host ----
                    imt = rows.tile([128, D], F32R, tag="imt")
                    nc.sync.dma_start(imt[:, :], im_r[tok0:tok0 + 128, :])
                    rin = stat.tile([128, 1], F32, tag="rin")
                    nc.sync.dma_start(rin[:, :], rin_in[tok0:tok0 + 128, :])

                    # ---- s tile token-major ----
                    ps_t = ps_small.tile([128, M_SLOTS], F32, tag="pss")
                    nc.tensor.transpose(ps_t[:, :], sr_sb[:, tsl], ident[:M_SLOTS, :M_SLOTS])
                    s_tok = small.tile([128, M_SLOTS], F32, tag="s_tok")
                    nc.vector.tensor_scalar_mul(s_tok[:, :], ps_t[:, :], rin[:, :])

                    # ---- top-1 mask & weights ----
                    rmax = aux2_acc[:, 3 * tile_idx:3 * tile_idx + 1]
                    nc.vector.tensor_reduce(rmax, s_tok[:, :], AX.X, ALU.max)
                    e_tok = small.tile([128, M_SLOTS], F32, tag="e_tok")
                    nc.scalar.activation(e_tok[:, :], s_tok[:, :], ACTF.Exp)
                    ind = small.tile([128, M_SLOTS], F32, tag="ind")
                    nc.vector.tensor_scalar(ind[:, :], s_tok[:, :], rmax, None,
                                            op0=ALU.is_ge)
                    # top-2 value and argmax index (for host-side near-tie fixup)
                    msk = small.tile([128, M_SLOTS], F32, tag="msk")
                    nc.vector.scalar_tensor_tensor(msk[:, :], ind[:, :], -1.0e30,
                                                   s_tok[:, :], ALU.mult, ALU.add)
                    nc.vector.tensor_reduce(aux2_acc[:, 3 * tile_idx + 1:3 * tile_idx + 2],
                                            msk[:, :], AX.X, ALU.max)
                    nc.vector.tensor_tensor(msk[:, :], ind[:, :], iota_sb[:, :], ALU.mult)
                    nc.vector.tensor_reduce(aux2_acc[:, 3 * tile_idx + 2:3 * tile_idx + 3],
                                            msk[:, :], AX.X, ALU.max)
                    er = small.tile([128, M_SLOTS], F32, tag="er")
                    nc.vector.tensor_scalar_mul(er[:, :], e_tok[:, :], rin[:, :])
                    wm = small.tile([128, M_SLOTS], F32R, tag="wm")
                    nc.vector.tensor_tensor(wm[:, :], er[:, :], ind[:, :], ALU.mult)
                    nc.vector.tensor_tensor(cmax_acc[:, :], cmax_acc[:, :], s_tok[:, :], ALU.max)
                    nc.vector.tensor_tensor(ind_acc[:, :], ind_acc[:, :], ind[:, :], ALU.add)

                    # ---- slot sums (segment-sum as matmul) ----
                    first = (tile_idx == 0)
                    last = (tile_idx == CL // 128 - 1)
                    for h in range(2):
                        nc.tensor.matmul(u_psum[:, h * 512:(h + 1) * 512], (wm[:, :]),
                                         (imt[:, h * 512:(h + 1) * 512]),
                                         start=first, stop=last)

                    # ---- extractor ----
                    t_sb = rows.tile([128, D], F32, tag="t_sb")
                    nc.sync.dma_start(t_sb[:, :], t_r[tok0:tok0 + 128, :])
                    tf_sb = rows.tile([128, D], F32, tag="tf_sb")
                    for h in range(2):
                        hsl = slice(h * 512, (h + 1) * 512)
                        tfp = ps_big.tile([128, 512], F32, tag="tfp")
                        for c in range(8):
                            nc.tensor.matmul(tfp[:, :], ttg[:, c, tsl], w1t_sb[:, c, hsl],
                                             start=(c == 0), stop=False)
                        nc.tensor.matmul(tfp[:, :], smT[:, tsl], g_sb[:, hsl],
                                         start=False, stop=True)
                        nc.vector.tensor_tensor(tf_sb[:, hsl], tfp[:, :], t_sb[:, hsl], ALU.add)
                    nc.sync.dma_start(tf_out[tok0:tok0 + 128, :], tf_sb[:, :])

                    # ---- loss ----
                    ss2 = stat.tile([128, 1], F32, tag="ss2")
                    sq_scr = work.tile([128, D], F32, tag="scr")
                    nc.scalar.activation(sq_scr[:, :], tf_sb[:, :], ACTF.Square,
                                         accum_out=ss2[:, :])
                    ln2 = stat.tile([128, 1], F32, tag="ln2")
                    nc.scalar.activation(ln2[:, :], ss2[:, :], ACTF.Ln)
                    rn2 = stat.tile([128, 1], F32, tag="rn2")
                    nc.scalar.activation(rn2[:, :], ln2[:, :], ACTF.Exp, scale=-0.5)
                    lntf = work.tile([128, D], F32, tag="lntf")
                    nc.vector.tensor_scalar_mul(lntf[:, :], tf_sb[:, :], rn2[:, :])
                    diff = work.tile([128, D], F32, tag="diff")
                    nc.gpsimd.tensor_tensor(diff[:, :], lntf[:, :], t_sb[:, :], ALU.subtract)
                    abs_scr = work.tile([128, D], F32, tag="scr")
                    nc.scalar.activation(abs_scr[:, :], diff[:, :], ACTF.Abs,
                                         accum_out=loss_cols[:, tile_idx:tile_idx + 1])

            # ---- finalize ----
            u_sb = wpool.tile([M_SLOTS, D], F32)
            nc.scalar.copy(u_sb[:, :], u_psum[:, :])
            nc.sync.dma_start(u_out[:, :], u_sb[:, :])
            aux_sb = accp.tile([128, 51], F32)
            nc.vector.tensor_copy(aux_sb[:, 0:M_SLOTS], ind_acc[:, :])
            nc.vector.tensor_copy(aux_sb[:, M_SLOTS:2 * M_SLOTS], cmax_acc[:, :])
            nc.vector.tensor_reduce(aux_sb[:, 50:51], loss_cols[:, :], AX.X, ALU.add)
            nc.sync.dma_start(aux_out[:, :], aux_sb[:, :])
            nc.sync.dma_start(aux2_out[:, :], aux2_acc[:, :])

    nc.compile()
    return nc


def _l2norm_np(x, axis=-1, eps=1e-12):
    n = np.linalg.norm(x, axis=axis, keepdims=True)
    return x / np.maximum(n, eps)


def make_in_maps(text_token, image_token, cache, W_ext):
    text_token = np.asarray(text_token, dtype=np.float32)
    image_token = np.asarray(image_token, dtype=np.float32)
    cache = np.asarray(cache, dtype=np.float32)
    W_ext = np.asarray(W_ext, dtype=np.float32)

    import ml_dtypes
    bf16 = ml_dtypes.bfloat16
    tT = np.ascontiguousarray(text_token.T.astype(bf16))
    iT = np.ascontiguousarray(image_token.T)
    w1t = np.ascontiguousarray((ALPHA * W_ext[:, :D]).T.astype(bf16))
    g = np.ascontiguousarray((ALPHA * (cache @ W_ext[:, D:].T)).astype(bf16))
    ct = np.ascontiguousarray(cache.T)
    ct_bf = ct.astype(bf16)
    ident = np.eye(128, dtype=np.float32)
    nrm = np.linalg.norm(image_token.astype(np.float64), axis=1, keepdims=True)
    rin_full = (1.0 / np.maximum(nrm, 1e-12)).astype(np.float32)

    in_maps = []
    for c in range(N_CORES):
        sl = slice(c * CL, (c + 1) * CL)
        in_maps.append({
            "tT": np.ascontiguousarray(tT[:, sl]),
            "t_r": np.ascontiguousarray(text_token[sl]),
            "iT": np.ascontiguousarray(iT[:, sl]),
            "im_r": np.ascontiguousarray(image_token[sl]),
            "w1t": w1t,
            "g_in": g,
            "ct_in": ct,
            "ct_bf_in": ct_bf,
            "ident_in": ident,
            "ones_in": np.ones((128, 32), dtype=np.float32),
            "iota_in": np.tile(np.arange(M_SLOTS, dtype=np.float32), (128, 1)),
            "rin_in": rin_full[sl],
        })
    return in_maps


GAP_THR = 1.0e-3


def combine_outputs(results, cache, image_token):
    """results: list of 8 dicts with tf_out/u_out/aux_out/aux2_out. Host unshard.

    Near-tie argmax fixup: the device's s is fp32-accurate but summation order
    differs from the reference; for tokens whose top-2 gap < GAP_THR, recompute
    the slot scores in float64 and move the contribution if the device picked a
    different slot than the (essentially exact) fp64 argmax.
    """
    cache = np.asarray(cache, dtype=np.float32)
    tf = np.concatenate([r["tf_out"] for r in results], axis=0)

    u_sum = np.sum([r["u_out"] for r in results], axis=0).astype(np.float64)  # [25, 1024]
    ind_sum = np.sum([r["aux_out"][:, :M_SLOTS] for r in results], axis=(0, 1)).astype(np.float64)
    col_max = np.max([r["aux_out"][:, M_SLOTS:2 * M_SLOTS] for r in results], axis=(0, 1))
    loss = np.sum([r["aux_out"][:, 50] for r in results]) / (C_FULL * D)

    cache64 = cache.astype(np.float64)
    image_token = np.asarray(image_token)
    for c, r in enumerate(results):
        a2 = r["aux2_out"].astype(np.float64)          # [128, 3*ntiles]
        top1 = a2[:, 0::3]                              # [128, 32]
        top2 = a2[:, 1::3]
        idx = a2[:, 2::3]
        amb = np.argwhere(top1 - top2 < GAP_THR)
        for p, t in amb:
            gi = c * CL + t * 128 + p
            imr = image_token[gi].astype(np.float64)
            bf = imr / max(np.linalg.norm(imr), 1e-12)
            srow = bf @ cache64.T
            j_true = int(np.argmax(srow))
            j_dev = int(round(idx[p, t]))
            if j_dev != j_true:
                u_sum[j_dev] -= np.exp(srow[j_dev]) * bf
                u_sum[j_true] += np.exp(srow[j_true]) * bf
                ind_sum[j_dev] -= 1.0
                ind_sum[j_true] += 1.0

    slot_sum = u_sum * np.exp(-col_max)[:, None]
    counts = ind_sum
    updated = np.where(counts[:, None] > 0,
                       MOMENTUM * cache + (1.0 - MOMENTUM) * slot_sum,
                       cache)
    new_cache = _l2norm_np(updated).astype(np.float32)
    return tf.astype(np.float32), np.float32(loss), new_cache


_NC_CACHE = {}


def kernel(text_token, image_token, cache, W_ext):
    if "nc" not in _NC_CACHE:
        _NC_CACHE["nc"] = build_nc()
    nc = _NC_CACHE["nc"]
    in_maps = make_in_maps(text_token, image_token, cache, W_ext)
    res = bass_utils.run_bass_kernel_spmd(nc, in_maps, core_ids=list(range(N_CORES)))
    return combine_outputs(res.results, cache, image_token)


# revision 19
# speedup vs baseline: 1.0731x; 1.0590x over previous
"""Trainium2 Bass kernel for the scatter_memory nn.Module problem.

Math (reference):
  read:  score = text @ cache.T; sm = softmax(score, axis=1); fine = sm @ cache
  ext:   tf = ALPHA * ([text, fine] @ W_ext.T) + text
  write: bf = l2norm(image); s = bf @ cache.T; sq = softmax(s, axis=0)
         assign = argmax(s, 1); w = sq[i, a_i] / colmax(sq)[a_i] = exp(s[i,a_i] - colmax(s)[a_i])
         slot_sum = segment_sum(w * bf, assign); counts = segment_sum(1, assign)
         new_cache = l2norm(where(counts>0, 0.8*cache + 0.2*slot_sum, cache))
  loss = mean |l2norm(tf) - text|

Sharding: data-parallel over tokens (C=32768 -> 4096/core on 8 cores).
Per-core outputs: tf shard, unnormalized slot sums u = sum_{i:a_i=j} exp(s_ij)*bf_i,
per-partition indicator sums / running col-max / loss partials. Host combines
(tiny [25,1024] work): slot_sum = exp(-M) * sum_c u_c, counts, new_cache, loss.

Device algebra:
  fine @ W2.T = sm @ (cache @ W2.T) = sm @ G  (G precomputed host-side, alpha-folded)
  tf = text @ (a*W1.T) + sm @ (a*G0) + text
"""

import numpy as np

import concourse.bass as bass
import concourse.bacc as bacc
import concourse.mybir as mybir
from concourse import tile
from concourse import bass_utils

F32 = mybir.dt.float32
F32R = mybir.dt.float32r  # fast fp32 matmul mode (rounded fp32)
BF16 = mybir.dt.bfloat16
AX = mybir.AxisListType
ALU = mybir.AluOpType
ACTF = mybir.ActivationFunctionType

C_FULL = 32768
D = 1024
M_SLOTS = 25
N_CORES = 8
CL = C_FULL // N_CORES          # 4096 tokens per core
GROUP = 512                     # tokens per pipeline group
N_GROUPS = CL // GROUP          # 8
SUBS = GROUP // 128             # 4 subtiles of 128 tokens per group
ALPHA = 0.2
MOMENTUM = 0.8




def build_nc():
    nc = bacc.Bacc("TRN2", target_bir_lowering=False, debug=False,
                   enable_asserts=False, num_devices=N_CORES)

    # ---- kernel I/O (per-core shard) ----
    tT = nc.dram_tensor("tT", [D, CL], BF16, kind="ExternalInput").ap()      # text.T shard
    t_r = nc.dram_tensor("t_r", [CL, D], F32, kind="ExternalInput").ap()    # text shard
    iT = nc.dram_tensor("iT", [D, CL], F32R, kind="ExternalInput").ap()      # image.T shard
    im_r = nc.dram_tensor("im_r", [CL, D], F32R, kind="ExternalInput").ap()  # image shard
    w1t = nc.dram_tensor("w1t", [D, D], BF16, kind="ExternalInput").ap()     # alpha*W1.T
    g_in = nc.dram_tensor("g_in", [M_SLOTS, D], BF16, kind="ExternalInput").ap()  # alpha*(cache@W2.T)
    ct_in = nc.dram_tensor("ct_in", [D, M_SLOTS], F32R, kind="ExternalInput").ap()  # cache.T fp32
    ct_bf_in = nc.dram_tensor("ct_bf_in", [D, M_SLOTS], BF16, kind="ExternalInput").ap()  # cache.T
    ident_in = nc.dram_tensor("ident_in", [128, 128], F32, kind="ExternalInput").ap()
    ones_in = nc.dram_tensor("ones_in", [128, 32], F32R, kind="ExternalInput").ap()
    iota_in = nc.dram_tensor("iota_in", [128, M_SLOTS], F32, kind="ExternalInput").ap()
    rin_in = nc.dram_tensor("rin_in", [CL, 1], F32, kind="ExternalInput").ap()  # 1/||image||

    tf_out = nc.dram_tensor("tf_out", [CL, D], F32, kind="ExternalOutput").ap()
    u_out = nc.dram_tensor("u_out", [M_SLOTS, D], F32, kind="ExternalOutput").ap()
    # aux: cols 0:25 indicator sums, 25:50 running col-max of s, 50 loss partial
    aux_out = nc.dram_tensor("aux_out", [128, 51], F32, kind="ExternalOutput").ap()
    # aux2: per token (p, tile t): [3t]=top1 of s, [3t+1]=top2, [3t+2]=argmax idx
    aux2_out = nc.dram_tensor("aux2_out", [128, 3 * CL // 128], F32, kind="ExternalOutput").ap()

    with tile.TileContext(nc) as tc, nc.allow_low_precision(reason="fp32r matmul operands are rounded fp32"):
        with (
            tc.tile_pool(name="const", bufs=1) as constp,
            tc.tile_pool(name="wpool", bufs=1) as wpool,
            tc.tile_pool(name="bigin", bufs=2) as bigin,
            tc.tile_pool(name="rows", bufs=2) as rows,
            tc.tile_pool(name="work", bufs=2) as work,
            tc.tile_pool(name="small", bufs=2) as small,
            tc.tile_pool(name="stat", bufs=6) as stat,
            tc.tile_pool(name="acc", bufs=1) as accp,
            tc.tile_pool(name="ps_small", bufs=3, space="PSUM") as ps_small,
            tc.tile_pool(name="ps_big", bufs=3, space="PSUM") as ps_big,
            tc.tile_pool(name="ps_u", bufs=1, space="PSUM") as ps_u,
        ):
            # ---- persistent SBUF ----
            w1t_sb = wpool.tile([128, 8, D], BF16)     # 32KB/part
            for c in range(8):
                nc.sync.dma_start(w1t_sb[:, c, :], w1t[c * 128:(c + 1) * 128, :])
            ct_sb = constp.tile([128, 8, M_SLOTS], F32R)
            for c in range(8):
                nc.sync.dma_start(ct_sb[:, c, :], ct_in[c * 128:(c + 1) * 128, :])
            ct_bf = constp.tile([128, 8, M_SLOTS], BF16)
            for c in range(8):
                nc.sync.dma_start(ct_bf[:, c, :], ct_bf_in[c * 128:(c + 1) * 128, :])
            g_sb = constp.tile([M_SLOTS, D], BF16)
            nc.sync.dma_start(g_sb[:, :], g_in[:, :])
            ident = constp.tile([128, 128], F32)
            nc.sync.dma_start(ident[:, :], ident_in[:, :])
            ones_sb = constp.tile([128, 32], F32R)
            nc.sync.dma_start(ones_sb[:, :], ones_in[:, :])
            iota_sb = constp.tile([128, M_SLOTS], F32)
            nc.sync.dma_start(iota_sb[:, :], iota_in[:, :])
            magic_sb = constp.tile([128, 1], mybir.dt.int32)
            nc.vector.memset(magic_sb[:, :], 0x5F3759DF)

            # ---- accumulators ----
            u_psum = ps_u.tile([M_SLOTS, D], F32)          # 2 banks, lives all loop
            ind_acc = accp.tile([128, M_SLOTS], F32)
            cmax_acc = accp.tile([128, M_SLOTS], F32)
            loss_cols = accp.tile([128, CL // 128], F32)
            aux2_acc = accp.tile([128, 3 * CL // 128], F32)
            nc.vector.memset(ind_acc[:, :], 0.0)
            nc.vector.memset(cmax_acc[:, :], -3.0e38)

            for grp in range(N_GROUPS):
                g0 = grp * GROUP

                # ---------- loads ----------
                ttg = bigin.tile([128, 8, GROUP], BF16, tag="ttg")
                nc.sync.dma_start(ttg[:, :, :],
                                  tT.rearrange("(c p) t -> p c t", p=128)[:, :, g0:g0 + GROUP])
                itg = bigin.tile([128, 8, GROUP], F32R, tag="itg")
                nc.sync.dma_start(itg[:, :, :],
                                  iT.rearrange("(c p) t -> p c t", p=128)[:, :, g0:g0 + GROUP])

                # ---------- read path: softmax over slots ----------
                scT = ps_small.tile([M_SLOTS, GROUP], F32, tag="pss")
                for c in range(8):
                    nc.tensor.matmul(scT[:, :], ct_bf[:, c, :], ttg[:, c, :],
                                     start=(c == 0), stop=(c == 7))
                escT = small.tile([M_SLOTS, GROUP], F32R, tag="escT")
                nc.scalar.activation(escT[:, :], scT[:, :], ACTF.Exp)
                s_sum = ps_small.tile([1, GROUP], F32, tag="pss")
                nc.tensor.matmul(s_sum[:, :], (ones_sb[:M_SLOTS, :1]), (escT[:, :]),
                                 start=True, stop=True)
                ln_s = small.tile([1, GROUP], F32, tag="lns")
                nc.scalar.activation(ln_s[:, :], s_sum[:, :], ACTF.Ln)
                r_sum = small.tile([1, GROUP], F32R, tag="rsum")
                nc.scalar.activation(r_sum[:, :], ln_s[:, :], ACTF.Exp, scale=-1.0)
                bcast = ps_small.tile([M_SLOTS, GROUP], F32, tag="pss")
                nc.tensor.matmul(bcast[:, :], (ones_sb[:1, :M_SLOTS]), (r_sum[:, :]),
                                 start=True, stop=True)
                smT = small.tile([M_SLOTS, GROUP], BF16, tag="smT")
                nc.vector.tensor_tensor(smT[:, :], escT[:, :], bcast[:, :], ALU.mult)

                # ---------- write path: raw scores ----------
                srT = ps_small.tile([M_SLOTS, GROUP], F32, tag="pss")
                for c in range(8):
                    nc.tensor.matmul(srT[:, :], ct_sb[:, c, :], itg[:, c, :],
                                     start=(c == 0), stop=(c == 7))
                sr_sb = small.tile([M_SLOTS, GROUP], F32, tag="sr_sb")
                nc.scalar.copy(sr_sb[:, :], srT[:, :])

                rin_g = stat.tile([128, SUBS], F32, tag="rin_g")
                nc.sync.dma_start(
                    rin_g[:, :],
                    rin_in.rearrange("(g s p) o -> p (g s o)", p=128, s=SUBS)[:, grp * SUBS:(grp + 1) * SUBS])
                imt_h = {}
                t_h = {}
                tf_h = {}
                for hh in range(2):
                    imt_hh = rows.tile([128, 2, D], F32R, tag="imt")
                    imt_h[hh] = imt_hh
                    nc.sync.dma_start(
                        imt_h[hh][:, :, :],
                        im_r.rearrange("(q p) d -> p q d", p=128)[:, grp * SUBS + hh * 2: grp * SUBS + hh * 2 + 2, :])
                    t_hh = rows.tile([128, 2, D], F32, tag="t_sb")
                    t_h[hh] = t_hh
                    nc.sync.dma_start(
                        t_h[hh][:, :, :],
                        t_r.rearrange("(q p) d -> p q d", p=128)[:, grp * SUBS + hh * 2: grp * SUBS + hh * 2 + 2, :])
                    tf_hh = rows.tile([128, 2, D], F32, tag="tf_sb")
                    tf_h[hh] = tf_hh

                for s in range(SUBS):
                    tok0 = g0 + s * 128
                    tsl = slice(s * 128, (s + 1) * 128)
                    tile_idx = tok0 // 128

                    # ---- normalized image: rin precomputed on host ----
                    imt = imt_h[s // 2][:, s % 2, :]
                    rin = rin_g[:, s:s + 1]

                    # ---- s tile token-major ----
                    ps_t = ps_small.tile([128, M_SLOTS], F32, tag="pss")
                    nc.tensor.transpose(ps_t[:, :], sr_sb[:, tsl], ident[:M_SLOTS, :M_SLOTS])
                    s_tok = small.tile([128, M_SLOTS], F32, tag="s_tok")
                    nc.vector.tensor_scalar_mul(s_tok[:, :], ps_t[:, :], rin[:, :])

                    # ---- top-1 mask & weights ----
                    rmax = aux2_acc[:, 3 * tile_idx:3 * tile_idx + 1]
                    nc.vector.tensor_reduce(rmax, s_tok[:, :], AX.X, ALU.max)
                    e_tok = small.tile([128, M_SLOTS], F32, tag="e_tok")
                    nc.scalar.activation(e_tok[:, :], s_tok[:, :], ACTF.Exp)
                    ind = small.tile([128, M_SLOTS], F32, tag="ind")
                    nc.vector.tensor_scalar(ind[:, :], s_tok[:, :], rmax, None,
                                            op0=ALU.is_ge)
                    # top-2 value and argmax index (for host-side near-tie fixup)
                    msk = small.tile([128, M_SLOTS], F32, tag="msk")
                    nc.vector.scalar_tensor_tensor(msk[:, :], ind[:, :], -1.0e30,
                                                   s_tok[:, :], ALU.mult, ALU.add)
                    nc.vector.tensor_reduce(aux2_acc[:, 3 * tile_idx + 1:3 * tile_idx + 2],
                                            msk[:, :], AX.X, ALU.max)
                    nc.vector.tensor_tensor(msk[:, :], ind[:, :], iota_sb[:, :], ALU.mult)
                    nc.vector.tensor_reduce(aux2_acc[:, 3 * tile_idx + 2:3 * tile_idx + 3],
                                            msk[:, :], AX.X, ALU.max)
                    er = small.tile([128, M_SLOTS], F32, tag="er")
                    nc.vector.tensor_scalar_mul(er[:, :], e_tok[:, :], rin[:, :])
                    wm = small.tile([128, M_SLOTS], F32R, tag="wm")
                    nc.vector.tensor_tensor(wm[:, :], er[:, :], ind[:, :], ALU.mult)
                    nc.vector.tensor_tensor(cmax_acc[:, :], cmax_acc[:, :], s_tok[:, :], ALU.max)
                    nc.vector.tensor_tensor(ind_acc[:, :], ind_acc[:, :], ind[:, :], ALU.add)

                    # ---- slot sums (segment-sum as matmul) ----
                    first = (tile_idx == 0)
                    last = (tile_idx == CL // 128 - 1)
                    for h in range(2):
                        nc.tensor.matmul(u_psum[:, h * 512:(h + 1) * 512], (wm[:, :]),
                                         (imt[:, h * 512:(h + 1) * 512]),
                                         start=first, stop=last)

                    # ---- extractor ----
                    t_sb = t_h[s // 2][:, s % 2, :]
                    tf_sb = tf_h[s // 2][:, s % 2, :]
                    for h in range(2):
                        hsl = slice(h * 512, (h + 1) * 512)
                        tfp = ps_big.tile([128, 512], F32, tag="tfp")
                        for c in range(8):
                            nc.tensor.matmul(tfp[:, :], ttg[:, c, tsl], w1t_sb[:, c, hsl],
                                             start=(c == 0), stop=False)
                        nc.tensor.matmul(tfp[:, :], smT[:, tsl], g_sb[:, hsl],
                                         start=False, stop=True)
                        nc.vector.tensor_tensor(tf_sb[:, hsl], tfp[:, :], t_sb[:, hsl], ALU.add)
                    if s % 2 == 1:
                        nc.sync.dma_start(
                            tf_out.rearrange("(q p) d -> p q d", p=128)[:, grp * SUBS + (s // 2) * 2: grp * SUBS + (s // 2) * 2 + 2, :],
                            tf_h[s // 2][:, :, :])

                    # ---- loss ----
                    ss2 = stat.tile([128, 1], F32, tag="ss2")
                    sq_scr = work.tile([128, D], F32, tag="scr")
                    nc.scalar.activation(sq_scr[:, :], tf_sb[:, :], ACTF.Square,
                                         accum_out=ss2[:, :])
                    # rn2 = rsqrt(ss2) via bit-hack seed + 3 Newton steps (DVE only,
                    # keeps the ACT engine inside one activation-table set)
                    rn2 = stat.tile([128, 1], F32, tag="rn2")
                    sh = stat.tile([128, 1], mybir.dt.int32, tag="sh")
                    nc.vector.tensor_scalar(sh[:, :], ss2[:, :].bitcast(mybir.dt.int32),
                                            1, None, op0=ALU.logical_shift_right)
                    nc.vector.scalar_tensor_tensor(rn2[:, :].bitcast(mybir.dt.int32),
                                                   sh[:, :], -1, magic_sb[:, :],
                                                   ALU.mult, ALU.add)
                    half = stat.tile([128, 1], F32, tag="half")
                    nc.vector.tensor_scalar(half[:, :], ss2[:, :], 0.5, None, op0=ALU.mult)
                    t1 = stat.tile([128, 1], F32, tag="t1")
                    for _ in range(3):
                        nc.vector.tensor_tensor(t1[:, :], rn2[:, :], rn2[:, :], ALU.mult)
                        nc.vector.tensor_tensor(t1[:, :], t1[:, :], half[:, :], ALU.mult)
                        nc.vector.tensor_scalar(t1[:, :], t1[:, :], 1.5, -1.0,
                                                op0=ALU.subtract, op1=ALU.mult)
                        nc.vector.tensor_tensor(rn2[:, :], rn2[:, :], t1[:, :], ALU.mult)
                    lntf = work.tile([128, D], F32, tag="lntf")
                    nc.vector.tensor_scalar_mul(lntf[:, :], tf_sb[:, :], rn2[:, :])
                    diff = work.tile([128, D], F32, tag="diff")
                    nc.gpsimd.tensor_tensor(diff[:, :], lntf[:, :], t_sb[:, :], ALU.subtract)
                    abs_scr = work.tile([128, D], F32, tag="scr")
                    nc.scalar.activation(abs_scr[:, :], diff[:, :], ACTF.Abs,
                                         accum_out=loss_cols[:, tile_idx:tile_idx + 1])

            # ---- finalize ----
            u_sb = wpool.tile([M_SLOTS, D], F32)
            nc.scalar.copy(u_sb[:, :], u_psum[:, :])
            nc.sync.dma_start(u_out[:, :], u_sb[:, :])
            aux_sb = accp.tile([128, 51], F32)
            nc.vector.tensor_copy(aux_sb[:, 0:M_SLOTS], ind_acc[:, :])
            nc.vector.tensor_copy(aux_sb[:, M_SLOTS:2 * M_SLOTS], cmax_acc[:, :])
            nc.vector.tensor_reduce(aux_sb[:, 50:51], loss_cols[:, :], AX.X, ALU.add)
            nc.sync.dma_start(aux_out[:, :], aux_sb[:, :])
            nc.sync.dma_start(aux2_out[:, :], aux2_acc[:, :])

    nc.compile()
    return nc


def _l2norm_np(x, axis=-1, eps=1e-12):
    n = np.linalg.norm(x, axis=axis, keepdims=True)
    return x / np.maximum(n, eps)


def make_in_maps(text_token, image_token, cache, W_ext):
    text_token = np.asarray(text_token, dtype=np.float32)
    image_token = np.asarray(image_token, dtype=np.float32)
    cache = np.asarray(cache, dtype=np.float32)
    W_ext = np.asarray(W_ext, dtype=np.float32)

    import ml_dtypes
    bf16 = ml_dtypes.bfloat16
    tT = np.ascontiguousarray(text_token.T.astype(bf16))
    iT = np.ascontiguousarray(image_token.T)
    w1t = np.ascontiguousarray((ALPHA * W_ext[:, :D]).T.astype(bf16))
    g = np.ascontiguousarray((ALPHA * (cache @ W_ext[:, D:].T)).astype(bf16))
    ct = np.ascontiguousarray(cache.T)
    ct_bf = ct.astype(bf16)
    ident = np.eye(128, dtype=np.float32)
    nrm = np.linalg.norm(image_token.astype(np.float64), axis=1, keepdims=True)
    rin_full = (1.0 / np.maximum(nrm, 1e-12)).astype(np.float32)

    in_maps = []
    for c in range(N_CORES):
        sl = slice(c * CL, (c + 1) * CL)
        in_maps.append({
            "tT": np.ascontiguousarray(tT[:, sl]),
            "t_r": np.ascontiguousarray(text_token[sl]),
            "iT": np.ascontiguousarray(iT[:, sl]),
            "im_r": np.ascontiguousarray(image_token[sl]),
            "w1t": w1t,
            "g_in": g,
            "ct_in": ct,
            "ct_bf_in": ct_bf,
            "ident_in": ident,
            "ones_in": np.ones((128, 32), dtype=np.float32),
            "iota_in": np.tile(np.arange(M_SLOTS, dtype=np.float32), (128, 1)),
            "rin_in": rin_full[sl],
        })
    return in_maps


GAP_THR = 1.0e-3


def combine_outputs(results, cache, image_token):
    """results: list of 8 dicts with tf_out/u_out/aux_out/aux2_out. Host unshard.

    Near-tie argmax fixup: the device's s is fp32-accurate but summation order
    differs from the reference; for tokens whose top-2 gap < GAP_THR, recompute
    the slot scores in float64 and move the contribution if the device picked a
    different slot than the (essentially exact) fp64 argmax.
    """
    cache = np.asarray(cache, dtype=np.float32)
    tf = np.concatenate([r["tf_out"] for r in results], axis=0)

    u_sum = np.sum([r["u_out"] for r in results], axis=0).astype(np.float64)  # [25, 1024]
    ind_sum = np.sum([r["aux_out"][:, :M_SLOTS] for r in results], axis=(0, 1)).astype(np.float64)
    col_max = np.max([r["aux_out"][:, M_SLOTS:2 * M_SLOTS] for r in results], axis=(0, 1))
    loss = np.sum([r["aux_out"][:, 50] for r in results]) / (C_FULL * D)

    cache64 = cache.astype(np.float64)
    image_token = np.asarray(image_token)
    for c, r in enumerate(results):
        a2 = r["aux2_out"].astype(np.float64)          # [128, 3*ntiles]
        top1 = a2[:, 0::3]                              # [128, 32]
        top2 = a2[:, 1::3]
        idx = a2[:, 2::3]
        amb = np.argwhere(top1 - top2 < GAP_THR)
        for p, t in amb:
            gi = c * CL + t * 128 + p
            imr = image_token[gi].astype(np.float64)
            bf = imr / max(np.linalg.norm(imr), 1e-12)
            srow = bf @ cache64.T
            j_true = int(np.argmax(srow))
            j_dev = int(round(idx[p, t]))
            if j_dev != j_true:
                u_sum[j_dev] -= np.exp(srow[j_dev]) * bf
                u_sum[j_true] += np.exp(srow[j_true]) * bf
                ind_sum[j_dev] -= 1.0
                ind_sum[j_true] += 1.0

    slot_sum = u_sum * np.exp(-col_max)[:, None]
    counts = ind_sum
    updated = np.where(counts[:, None] > 0,
                       MOMENTUM * cache + (1.0 - MOMENTUM) * slot_sum,
                       cache)
    new_cache = _l2norm_np(updated).astype(np.float32)
    return tf.astype(np.float32), np.float32(loss), new_cache


_NC_CACHE = {}


def kernel(text_token, image_token, cache, W_ext):
    if "nc" not in _NC_CACHE:
        _NC_CACHE["nc"] = build_nc()
    nc = _NC_CACHE["nc"]
    in_maps = make_in_maps(text_token, image_token, cache, W_ext)
    res = bass_utils.run_bass_kernel_spmd(nc, in_maps, core_ids=list(range(N_CORES)))
    return combine_outputs(res.results, cache, image_token)


# revision 21
# speedup vs baseline: 1.1712x; 1.0914x over previous
"""Trainium2 Bass kernel for the scatter_memory nn.Module problem.

Math (reference):
  read:  score = text @ cache.T; sm = softmax(score, axis=1); fine = sm @ cache
  ext:   tf = ALPHA * ([text, fine] @ W_ext.T) + text
  write: bf = l2norm(image); s = bf @ cache.T; sq = softmax(s, axis=0)
         assign = argmax(s, 1); w = sq[i, a_i] / colmax(sq)[a_i] = exp(s[i,a_i] - colmax(s)[a_i])
         slot_sum = segment_sum(w * bf, assign); counts = segment_sum(1, assign)
         new_cache = l2norm(where(counts>0, 0.8*cache + 0.2*slot_sum, cache))
  loss = mean |l2norm(tf) - text|

Sharding: data-parallel over tokens (C=32768 -> 4096/core on 8 cores).
Per-core outputs: tf shard, unnormalized slot sums u = sum_{i:a_i=j} exp(s_ij)*bf_i,
per-partition indicator sums / running col-max / loss partials. Host combines
(tiny [25,1024] work): slot_sum = exp(-M) * sum_c u_c, counts, new_cache, loss.

Device algebra:
  fine @ W2.T = sm @ (cache @ W2.T) = sm @ G  (G precomputed host-side, alpha-folded)
  tf = text @ (a*W1.T) + sm @ (a*G0) + text
"""

import numpy as np

import concourse.bass as bass
import concourse.bacc as bacc
import concourse.mybir as mybir
from concourse import tile
from concourse import bass_utils

F32 = mybir.dt.float32
F32R = mybir.dt.float32r  # fast fp32 matmul mode (rounded fp32)
BF16 = mybir.dt.bfloat16
AX = mybir.AxisListType
ALU = mybir.AluOpType
ACTF = mybir.ActivationFunctionType

C_FULL = 32768
D = 1024
M_SLOTS = 25
N_CORES = 8
CL = C_FULL // N_CORES          # 4096 tokens per core
GROUP = 512                     # tokens per pipeline group
N_GROUPS = CL // GROUP          # 8
SUBS = GROUP // 128             # 4 subtiles of 128 tokens per group
ALPHA = 0.2
MOMENTUM = 0.8




def build_nc():
    nc = bacc.Bacc("TRN2", target_bir_lowering=False, debug=False,
                   enable_asserts=False, num_devices=N_CORES)

    # ---- kernel I/O (per-core shard) ----
    tT = nc.dram_tensor("tT", [D, CL], BF16, kind="ExternalInput").ap()      # text.T shard
    t_r = nc.dram_tensor("t_r", [CL, D], F32, kind="ExternalInput").ap()    # text shard
    iT = nc.dram_tensor("iT", [D, CL], F32R, kind="ExternalInput").ap()      # image.T shard
    im_r = nc.dram_tensor("im_r", [CL, D], F32R, kind="ExternalInput").ap()  # image shard
    w1t = nc.dram_tensor("w1t", [D, D], BF16, kind="ExternalInput").ap()     # alpha*W1.T
    g_in = nc.dram_tensor("g_in", [M_SLOTS, D], BF16, kind="ExternalInput").ap()  # alpha*(cache@W2.T)
    ct_in = nc.dram_tensor("ct_in", [D, M_SLOTS], F32R, kind="ExternalInput").ap()  # cache.T fp32
    ct_bf_in = nc.dram_tensor("ct_bf_in", [D, M_SLOTS], BF16, kind="ExternalInput").ap()  # cache.T
    ident_in = nc.dram_tensor("ident_in", [128, 128], F32, kind="ExternalInput").ap()
    ones_in = nc.dram_tensor("ones_in", [128, 32], F32R, kind="ExternalInput").ap()
    iota_in = nc.dram_tensor("iota_in", [128, M_SLOTS], F32, kind="ExternalInput").ap()
    rin_in = nc.dram_tensor("rin_in", [CL, 1], F32, kind="ExternalInput").ap()  # 1/||image||

    tf_out = nc.dram_tensor("tf_out", [CL, D], F32, kind="ExternalOutput").ap()
    u_out = nc.dram_tensor("u_out", [M_SLOTS, D], F32, kind="ExternalOutput").ap()
    # aux: cols 0:25 indicator sums, 25:50 running col-max of s, 50 loss partial
    aux_out = nc.dram_tensor("aux_out", [128, 51], F32, kind="ExternalOutput").ap()
    # aux2: per token (p, tile t): [3t]=top1 of s, [3t+1]=top2, [3t+2]=argmax idx
    aux2_out = nc.dram_tensor("aux2_out", [128, 3 * CL // 128], F32, kind="ExternalOutput").ap()

    with tile.TileContext(nc) as tc, nc.allow_low_precision(reason="fp32r matmul operands are rounded fp32"):
        with (
            tc.tile_pool(name="const", bufs=1) as constp,
            tc.tile_pool(name="wpool", bufs=1) as wpool,
            tc.tile_pool(name="bigin", bufs=2) as bigin,
            tc.tile_pool(name="rows", bufs=3) as rows,
            tc.tile_pool(name="work", bufs=2) as work,
            tc.tile_pool(name="small", bufs=2) as small,
            tc.tile_pool(name="stat", bufs=6) as stat,
            tc.tile_pool(name="acc", bufs=1) as accp,
            tc.tile_pool(name="ps_small", bufs=3, space="PSUM") as ps_small,
            tc.tile_pool(name="ps_big", bufs=3, space="PSUM") as ps_big,
            tc.tile_pool(name="ps_u", bufs=1, space="PSUM") as ps_u,
        ):
            # ---- persistent SBUF ----
            w1t_sb = wpool.tile([128, 8, D], BF16)     # 32KB/part
            for c in range(8):
                nc.sync.dma_start(w1t_sb[:, c, :], w1t[c * 128:(c + 1) * 128, :])
            ct_sb = constp.tile([128, 8, M_SLOTS], F32R)
            for c in range(8):
                nc.sync.dma_start(ct_sb[:, c, :], ct_in[c * 128:(c + 1) * 128, :])
            ct_bf = constp.tile([128, 8, M_SLOTS], BF16)
            for c in range(8):
                nc.sync.dma_start(ct_bf[:, c, :], ct_bf_in[c * 128:(c + 1) * 128, :])
            g_sb = constp.tile([M_SLOTS, D], BF16)
            nc.sync.dma_start(g_sb[:, :], g_in[:, :])
            ident = constp.tile([128, 128], F32)
            nc.sync.dma_start(ident[:, :], ident_in[:, :])
            ones_sb = constp.tile([128, 32], F32R)
            nc.sync.dma_start(ones_sb[:, :], ones_in[:, :])
            iota_sb = constp.tile([128, M_SLOTS], F32)
            nc.sync.dma_start(iota_sb[:, :], iota_in[:, :])
            magicb = constp.tile([128, SUBS], mybir.dt.int32)
            nc.vector.memset(magicb[:, :], 0x5F3759DF)

            # ---- accumulators ----
            u_psum = ps_u.tile([M_SLOTS, D], F32)          # 2 banks, lives all loop
            ind_acc = accp.tile([128, M_SLOTS], F32)
            cmax_acc = accp.tile([128, M_SLOTS], F32)
            loss_cols = accp.tile([128, CL // 128], F32)
            aux2_acc = accp.tile([128, 3 * CL // 128], F32)
            nc.vector.memset(ind_acc[:, :], 0.0)
            nc.vector.memset(cmax_acc[:, :], -3.0e38)

            for grp in range(N_GROUPS):
                g0 = grp * GROUP

                # ---------- loads ----------
                ttg = bigin.tile([128, 8, GROUP], BF16, tag="ttg")
                nc.sync.dma_start(ttg[:, :, :],
                                  tT.rearrange("(c p) t -> p c t", p=128)[:, :, g0:g0 + GROUP])
                itg = bigin.tile([128, 8, GROUP], F32R, tag="itg")
                nc.sync.dma_start(itg[:, :, :],
                                  iT.rearrange("(c p) t -> p c t", p=128)[:, :, g0:g0 + GROUP])

                # ---------- read path: softmax over slots ----------
                scT = ps_small.tile([M_SLOTS, GROUP], F32, tag="pss")
                for c in range(8):
                    nc.tensor.matmul(scT[:, :], ct_bf[:, c, :], ttg[:, c, :],
                                     start=(c == 0), stop=(c == 7))
                escT = small.tile([M_SLOTS, GROUP], F32R, tag="escT")
                nc.scalar.activation(escT[:, :], scT[:, :], ACTF.Exp)
                s_sum = ps_small.tile([1, GROUP], F32, tag="pss")
                nc.tensor.matmul(s_sum[:, :], (ones_sb[:M_SLOTS, :1]), (escT[:, :]),
                                 start=True, stop=True)
                ln_s = small.tile([1, GROUP], F32, tag="lns")
                nc.scalar.activation(ln_s[:, :], s_sum[:, :], ACTF.Ln)
                r_sum = small.tile([1, GROUP], F32R, tag="rsum")
                nc.scalar.activation(r_sum[:, :], ln_s[:, :], ACTF.Exp, scale=-1.0)
                bcast = ps_small.tile([M_SLOTS, GROUP], F32, tag="pss")
                nc.tensor.matmul(bcast[:, :], (ones_sb[:1, :M_SLOTS]), (r_sum[:, :]),
                                 start=True, stop=True)
                smT = small.tile([M_SLOTS, GROUP], BF16, tag="smT")
                nc.vector.tensor_tensor(smT[:, :], escT[:, :], bcast[:, :], ALU.mult)

                # ---------- write path: raw scores ----------
                srT = ps_small.tile([M_SLOTS, GROUP], F32, tag="pss")
                for c in range(8):
                    nc.tensor.matmul(srT[:, :], ct_sb[:, c, :], itg[:, c, :],
                                     start=(c == 0), stop=(c == 7))
                sr_sb = small.tile([M_SLOTS, GROUP], F32, tag="sr_sb")
                nc.scalar.copy(sr_sb[:, :], srT[:, :])

                rin_g = stat.tile([128, SUBS], F32, tag="rin_g")
                nc.sync.dma_start(
                    rin_g[:, :],
                    rin_in.rearrange("(g s p) o -> p (g s o)", p=128, s=SUBS)[:, grp * SUBS:(grp + 1) * SUBS])
                imt_h = {}
                t_h = {}
                tf_h = {}
                for hh in range(2):
                    imt_hh = rows.tile([128, 2, D], F32R, tag="imt")
                    imt_h[hh] = imt_hh
                    nc.sync.dma_start(
                        imt_h[hh][:, :, :],
                        im_r.rearrange("(q p) d -> p q d", p=128)[:, grp * SUBS + hh * 2: grp * SUBS + hh * 2 + 2, :])
                    t_hh = rows.tile([128, 2, D], F32, tag="t_sb")
                    t_h[hh] = t_hh
                    nc.sync.dma_start(
                        t_h[hh][:, :, :],
                        t_r.rearrange("(q p) d -> p q d", p=128)[:, grp * SUBS + hh * 2: grp * SUBS + hh * 2 + 2, :])
                    tf_hh = rows.tile([128, 2, D], F32, tag="tf_sb")
                    tf_h[hh] = tf_hh

                ss2g = stat.tile([128, SUBS], F32, tag="ss2g")
                rn2g = stat.tile([128, SUBS], F32, tag="rn2g")

                for s in range(SUBS):
                    tok0 = g0 + s * 128
                    tsl = slice(s * 128, (s + 1) * 128)
                    tile_idx = tok0 // 128

                    # ---- normalized image: rin precomputed on host ----
                    imt = imt_h[s // 2][:, s % 2, :]
                    rin = rin_g[:, s:s + 1]

                    # ---- s tile token-major ----
                    ps_t = ps_small.tile([128, M_SLOTS], F32, tag="pss")
                    nc.tensor.transpose(ps_t[:, :], sr_sb[:, tsl], ident[:M_SLOTS, :M_SLOTS])
                    s_tok = small.tile([128, M_SLOTS], F32, tag="s_tok")
                    nc.vector.tensor_scalar_mul(s_tok[:, :], ps_t[:, :], rin[:, :])

                    # ---- top-1 mask & weights ----
                    rmax = aux2_acc[:, 3 * tile_idx:3 * tile_idx + 1]
                    nc.vector.tensor_reduce(rmax, s_tok[:, :], AX.X, ALU.max)
                    e_tok = small.tile([128, M_SLOTS], F32, tag="e_tok")
                    nc.scalar.activation(e_tok[:, :], s_tok[:, :], ACTF.Exp)
                    ind = small.tile([128, M_SLOTS], F32, tag="ind")
                    nc.vector.tensor_scalar(ind[:, :], s_tok[:, :], rmax, None,
                                            op0=ALU.is_ge)
                    # top-2 value and argmax index (for host-side near-tie fixup)
                    msk = small.tile([128, M_SLOTS], F32, tag="msk")
                    nc.vector.scalar_tensor_tensor(msk[:, :], ind[:, :], -1.0e30,
                                                   s_tok[:, :], ALU.mult, ALU.add)
                    nc.vector.tensor_reduce(aux2_acc[:, 3 * tile_idx + 1:3 * tile_idx + 2],
                                            msk[:, :], AX.X, ALU.max)
                    nc.vector.tensor_tensor(msk[:, :], ind[:, :], iota_sb[:, :], ALU.mult)
                    nc.vector.tensor_reduce(aux2_acc[:, 3 * tile_idx + 2:3 * tile_idx + 3],
                                            msk[:, :], AX.X, ALU.max)
                    er = small.tile([128, M_SLOTS], F32, tag="er")
                    nc.vector.tensor_scalar_mul(er[:, :], e_tok[:, :], rin[:, :])
                    wm = small.tile([128, M_SLOTS], F32R, tag="wm")
                    nc.vector.tensor_tensor(wm[:, :], er[:, :], ind[:, :], ALU.mult)
                    nc.vector.tensor_tensor(cmax_acc[:, :], cmax_acc[:, :], s_tok[:, :], ALU.max)
                    nc.vector.tensor_tensor(ind_acc[:, :], ind_acc[:, :], ind[:, :], ALU.add)

                    # ---- slot sums (segment-sum as matmul) ----
                    first = (tile_idx == 0)
                    last = (tile_idx == CL // 128 - 1)
                    for h in range(2):
                        nc.tensor.matmul(u_psum[:, h * 512:(h + 1) * 512], (wm[:, :]),
                                         (imt[:, h * 512:(h + 1) * 512]),
                                         start=first, stop=last)

                    # ---- extractor ----
                    t_sb = t_h[s // 2][:, s % 2, :]
                    tf_sb = tf_h[s // 2][:, s % 2, :]
                    for h in range(2):
                        hsl = slice(h * 512, (h + 1) * 512)
                        tfp = ps_big.tile([128, 512], F32, tag="tfp")
                        for c in range(8):
                            nc.tensor.matmul(tfp[:, :], ttg[:, c, tsl], w1t_sb[:, c, hsl],
                                             start=(c == 0), stop=False)
                        nc.tensor.matmul(tfp[:, :], smT[:, tsl], g_sb[:, hsl],
                                         start=False, stop=True)
                        nc.vector.tensor_tensor(tf_sb[:, hsl], tfp[:, :], t_sb[:, hsl], ALU.add)
                    if s % 2 == 1:
                        nc.sync.dma_start(
                            tf_out.rearrange("(q p) d -> p q d", p=128)[:, grp * SUBS + (s // 2) * 2: grp * SUBS + (s // 2) * 2 + 2, :],
                            tf_h[s // 2][:, :, :])

                    # ---- loss: row sum of tf^2 into the group stat tile ----
                    sq_scr = work.tile([128, D], F32, tag="scr")
                    nc.scalar.activation(sq_scr[:, :], tf_sb[:, :], ACTF.Square,
                                         accum_out=ss2g[:, s:s + 1])

                # ---- batched rn2 = rsqrt(ss2g): bit-hack seed + 3 Newton steps
                # (DVE-only, keeps ACT inside one activation-table set) ----
                sh = stat.tile([128, SUBS], mybir.dt.int32, tag="sh")
                nc.vector.tensor_scalar(sh[:, :], ss2g[:, :].bitcast(mybir.dt.int32),
                                        1, None, op0=ALU.logical_shift_right)
                nc.vector.scalar_tensor_tensor(rn2g[:, :].bitcast(mybir.dt.int32),
                                               sh[:, :], -1, magicb[:, :],
                                               ALU.mult, ALU.add)
                half = stat.tile([128, SUBS], F32, tag="half")
                nc.vector.tensor_scalar(half[:, :], ss2g[:, :], 0.5, None, op0=ALU.mult)
                t1 = stat.tile([128, SUBS], F32, tag="t1")
                for _ in range(3):
                    nc.vector.tensor_tensor(t1[:, :], rn2g[:, :], rn2g[:, :], ALU.mult)
                    nc.vector.tensor_tensor(t1[:, :], t1[:, :], half[:, :], ALU.mult)
                    nc.vector.tensor_scalar(t1[:, :], t1[:, :], 1.5, -1.0,
                                            op0=ALU.subtract, op1=ALU.mult)
                    nc.vector.tensor_tensor(rn2g[:, :], rn2g[:, :], t1[:, :], ALU.mult)

                for s in range(SUBS):
                    tile_idx = (grp * GROUP + s * 128) // 128
                    t_sb = t_h[s // 2][:, s % 2, :]
                    tf_sb = tf_h[s // 2][:, s % 2, :]
                    lntf = work.tile([128, D], F32, tag="lntf")
                    nc.scalar.activation(lntf[:, :], tf_sb[:, :], ACTF.Copy,
                                         scale=rn2g[:, s:s + 1])
                    nc.gpsimd.tensor_tensor(lntf[:, :], lntf[:, :], t_sb[:, :], ALU.subtract)
                    abs_scr = work.tile([128, D], F32, tag="scr")
                    nc.scalar.activation(abs_scr[:, :], lntf[:, :], ACTF.Abs,
                                         accum_out=loss_cols[:, tile_idx:tile_idx + 1])

            # ---- finalize ----
            u_sb = wpool.tile([M_SLOTS, D], F32)
            nc.scalar.copy(u_sb[:, :], u_psum[:, :])
            nc.sync.dma_start(u_out[:, :], u_sb[:, :])
            aux_sb = accp.tile([128, 51], F32)
            nc.vector.tensor_copy(aux_sb[:, 0:M_SLOTS], ind_acc[:, :])
            nc.vector.tensor_copy(aux_sb[:, M_SLOTS:2 * M_SLOTS], cmax_acc[:, :])
            nc.vector.tensor_reduce(aux_sb[:, 50:51], loss_cols[:, :], AX.X, ALU.add)
            nc.sync.dma_start(aux_out[:, :], aux_sb[:, :])
            nc.sync.dma_start(aux2_out[:, :], aux2_acc[:, :])

    nc.compile()
    return nc


def _l2norm_np(x, axis=-1, eps=1e-12):
    n = np.linalg.norm(x, axis=axis, keepdims=True)
    return x / np.maximum(n, eps)


def make_in_maps(text_token, image_token, cache, W_ext):
    text_token = np.asarray(text_token, dtype=np.float32)
    image_token = np.asarray(image_token, dtype=np.float32)
    cache = np.asarray(cache, dtype=np.float32)
    W_ext = np.asarray(W_ext, dtype=np.float32)

    import ml_dtypes
    bf16 = ml_dtypes.bfloat16
    tT = np.ascontiguousarray(text_token.T.astype(bf16))
    iT = np.ascontiguousarray(image_token.T)
    w1t = np.ascontiguousarray((ALPHA * W_ext[:, :D]).T.astype(bf16))
    g = np.ascontiguousarray((ALPHA * (cache @ W_ext[:, D:].T)).astype(bf16))
    ct = np.ascontiguousarray(cache.T)
    ct_bf = ct.astype(bf16)
    ident = np.eye(128, dtype=np.float32)
    nrm = np.linalg.norm(image_token.astype(np.float64), axis=1, keepdims=True)
    rin_full = (1.0 / np.maximum(nrm, 1e-12)).astype(np.float32)

    in_maps = []
    for c in range(N_CORES):
        sl = slice(c * CL, (c + 1) * CL)
        in_maps.append({
            "tT": np.ascontiguousarray(tT[:, sl]),
            "t_r": np.ascontiguousarray(text_token[sl]),
            "iT": np.ascontiguousarray(iT[:, sl]),
            "im_r": np.ascontiguousarray(image_token[sl]),
            "w1t": w1t,
            "g_in": g,
            "ct_in": ct,
            "ct_bf_in": ct_bf,
            "ident_in": ident,
            "ones_in": np.ones((128, 32), dtype=np.float32),
            "iota_in": np.tile(np.arange(M_SLOTS, dtype=np.float32), (128, 1)),
            "rin_in": rin_full[sl],
        })
    return in_maps


GAP_THR = 1.0e-3


def combine_outputs(results, cache, image_token):
    """results: list of 8 dicts with tf_out/u_out/aux_out/aux2_out. Host unshard.

    Near-tie argmax fixup: the device's s is fp32-accurate but summation order
    differs from the reference; for tokens whose top-2 gap < GAP_THR, recompute
    the slot scores in float64 and move the contribution if the device picked a
    different slot than the (essentially exact) fp64 argmax.
    """
    cache = np.asarray(cache, dtype=np.float32)
    tf = np.concatenate([r["tf_out"] for r in results], axis=0)

    u_sum = np.sum([r["u_out"] for r in results], axis=0).astype(np.float64)  # [25, 1024]
    ind_sum = np.sum([r["aux_out"][:, :M_SLOTS] for r in results], axis=(0, 1)).astype(np.float64)
    col_max = np.max([r["aux_out"][:, M_SLOTS:2 * M_SLOTS] for r in results], axis=(0, 1))
    loss = np.sum([r["aux_out"][:, 50] for r in results]) / (C_FULL * D)

    cache64 = cache.astype(np.float64)
    image_token = np.asarray(image_token)
    for c, r in enumerate(results):
        a2 = r["aux2_out"].astype(np.float64)          # [128, 3*ntiles]
        top1 = a2[:, 0::3]                              # [128, 32]
        top2 = a2[:, 1::3]
        idx = a2[:, 2::3]
        amb = np.argwhere(top1 - top2 < GAP_THR)
        for p, t in amb:
            gi = c * CL + t * 128 + p
            imr = image_token[gi].astype(np.float64)
            bf = imr / max(np.linalg.norm(imr), 1e-12)
            srow = bf @ cache64.T
            j_true = int(np.argmax(srow))
            j_dev = int(round(idx[p, t]))
            if j_dev != j_true:
                u_sum[j_dev] -= np.exp(srow[j_dev]) * bf
                u_sum[j_true] += np.exp(srow[j_true]) * bf
                ind_sum[j_dev] -= 1.0
                ind_sum[j_true] += 1.0

    slot_sum = u_sum * np.exp(-col_max)[:, None]
    counts = ind_sum
    updated = np.where(counts[:, None] > 0,
                       MOMENTUM * cache + (1.0 - MOMENTUM) * slot_sum,
                       cache)
    new_cache = _l2norm_np(updated).astype(np.float32)
    return tf.astype(np.float32), np.float32(loss), new_cache


_NC_CACHE = {}


def kernel(text_token, image_token, cache, W_ext):
    if "nc" not in _NC_CACHE:
        _NC_CACHE["nc"] = build_nc()
    nc = _NC_CACHE["nc"]
    in_maps = make_in_maps(text_token, image_token, cache, W_ext)
    res = bass_utils.run_bass_kernel_spmd(nc, in_maps, core_ids=list(range(N_CORES)))
    return combine_outputs(res.results, cache, image_token)


# revision 22
# speedup vs baseline: 1.4089x; 1.2030x over previous
"""Trainium2 Bass kernel for the scatter_memory nn.Module problem.

Math (reference):
  read:  score = text @ cache.T; sm = softmax(score, axis=1); fine = sm @ cache
  ext:   tf = ALPHA * ([text, fine] @ W_ext.T) + text
  write: bf = l2norm(image); s = bf @ cache.T; sq = softmax(s, axis=0)
         assign = argmax(s, 1); w = sq[i, a_i] / colmax(sq)[a_i] = exp(s[i,a_i] - colmax(s)[a_i])
         slot_sum = segment_sum(w * bf, assign); counts = segment_sum(1, assign)
         new_cache = l2norm(where(counts>0, 0.8*cache + 0.2*slot_sum, cache))
  loss = mean |l2norm(tf) - text|

Sharding: data-parallel over tokens (C=32768 -> 4096/core on 8 cores).
Per-core outputs: tf shard, unnormalized slot sums u = sum_{i:a_i=j} exp(s_ij)*bf_i,
per-partition indicator sums / running col-max / loss partials. Host combines
(tiny [25,1024] work): slot_sum = exp(-M) * sum_c u_c, counts, new_cache, loss.

Device algebra:
  fine @ W2.T = sm @ (cache @ W2.T) = sm @ G  (G precomputed host-side, alpha-folded)
  tf = text @ (a*W1.T) + sm @ (a*G0) + text
"""

import numpy as np

import concourse.bass as bass
import concourse.bacc as bacc
import concourse.mybir as mybir
from concourse import tile
from concourse import bass_utils

F32 = mybir.dt.float32
F32R = mybir.dt.float32r  # fast fp32 matmul mode (rounded fp32)
BF16 = mybir.dt.bfloat16
AX = mybir.AxisListType
ALU = mybir.AluOpType
ACTF = mybir.ActivationFunctionType

C_FULL = 32768
D = 1024
M_SLOTS = 25
N_CORES = 8
CL = C_FULL // N_CORES          # 4096 tokens per core
GROUP = 512                     # tokens per pipeline group
N_GROUPS = CL // GROUP          # 8
SUBS = GROUP // 128             # 4 subtiles of 128 tokens per group
ALPHA = 0.2
MOMENTUM = 0.8




def build_nc():
    nc = bacc.Bacc("TRN2", target_bir_lowering=False, debug=False,
                   enable_asserts=False, num_devices=N_CORES)

    # ---- kernel I/O (per-core shard) ----
    tT = nc.dram_tensor("tT", [D, CL], BF16, kind="ExternalInput").ap()      # text.T shard
    t_r = nc.dram_tensor("t_r", [CL, D], F32, kind="ExternalInput").ap()    # text shard
    iT = nc.dram_tensor("iT", [D, CL], F32R, kind="ExternalInput").ap()      # image.T shard
    im_r = nc.dram_tensor("im_r", [CL, D], F32R, kind="ExternalInput").ap()  # image shard
    w1t = nc.dram_tensor("w1t", [D, D], BF16, kind="ExternalInput").ap()     # alpha*W1.T
    g_in = nc.dram_tensor("g_in", [M_SLOTS, D], BF16, kind="ExternalInput").ap()  # alpha*(cache@W2.T)
    ct_in = nc.dram_tensor("ct_in", [D, M_SLOTS], F32R, kind="ExternalInput").ap()  # cache.T fp32
    ct_bf_in = nc.dram_tensor("ct_bf_in", [D, M_SLOTS], BF16, kind="ExternalInput").ap()  # cache.T
    ident_in = nc.dram_tensor("ident_in", [128, 128], F32, kind="ExternalInput").ap()
    ones_in = nc.dram_tensor("ones_in", [128, 32], F32R, kind="ExternalInput").ap()
    iota_in = nc.dram_tensor("iota_in", [128, M_SLOTS], F32, kind="ExternalInput").ap()
    rin_in = nc.dram_tensor("rin_in", [CL, 1], F32, kind="ExternalInput").ap()  # 1/||image||

    tf_out = nc.dram_tensor("tf_out", [CL, D], F32, kind="ExternalOutput").ap()
    u_out = nc.dram_tensor("u_out", [M_SLOTS, D], F32, kind="ExternalOutput").ap()
    # aux: cols 0:25 indicator sums, 25:50 running col-max of s, 50 loss partial
    aux_out = nc.dram_tensor("aux_out", [128, 51], F32, kind="ExternalOutput").ap()
    # aux2: per token (p, tile t): [3t]=top1 of s, [3t+1]=top2, [3t+2]=argmax idx
    aux2_out = nc.dram_tensor("aux2_out", [128, 3 * CL // 128], F32, kind="ExternalOutput").ap()

    with tile.TileContext(nc) as tc, nc.allow_low_precision(reason="fp32r matmul operands are rounded fp32"):
        with (
            tc.tile_pool(name="const", bufs=1) as constp,
            tc.tile_pool(name="wpool", bufs=1) as wpool,
            tc.tile_pool(name="bigin", bufs=2) as bigin,
            tc.tile_pool(name="rows", bufs=3) as rows,
            tc.tile_pool(name="work", bufs=2) as work,
            tc.tile_pool(name="small", bufs=2) as small,
            tc.tile_pool(name="stat", bufs=6) as stat,
            tc.tile_pool(name="acc", bufs=1) as accp,
            tc.tile_pool(name="ps_small", bufs=2, space="PSUM") as ps_small,
            tc.tile_pool(name="ps_big", bufs=2, space="PSUM") as ps_big,
            tc.tile_pool(name="ps_u", bufs=1, space="PSUM") as ps_u,
        ):
            # ---- persistent SBUF ----
            w1t_sb = wpool.tile([128, 8, D], BF16)     # 32KB/part
            for c in range(8):
                nc.sync.dma_start(w1t_sb[:, c, :], w1t[c * 128:(c + 1) * 128, :])
            ct_sb = constp.tile([128, 8, M_SLOTS], F32R)
            for c in range(8):
                nc.sync.dma_start(ct_sb[:, c, :], ct_in[c * 128:(c + 1) * 128, :])
            ct_bf = constp.tile([128, 8, M_SLOTS], BF16)
            for c in range(8):
                nc.sync.dma_start(ct_bf[:, c, :], ct_bf_in[c * 128:(c + 1) * 128, :])
            g_sb = constp.tile([M_SLOTS, D], BF16)
            nc.sync.dma_start(g_sb[:, :], g_in[:, :])
            ident = constp.tile([128, 128], F32)
            nc.sync.dma_start(ident[:, :], ident_in[:, :])
            ones_sb = constp.tile([128, 32], F32R)
            nc.sync.dma_start(ones_sb[:, :], ones_in[:, :])
            iota_sb = constp.tile([128, M_SLOTS], F32)
            nc.sync.dma_start(iota_sb[:, :], iota_in[:, :])
            magicb = constp.tile([128, SUBS], mybir.dt.int32)
            nc.vector.memset(magicb[:, :], 0x5F3759DF)

            # ---- accumulators ----
            u_psum = ps_u.tile([M_SLOTS, D], F32)          # 2 banks, lives all loop
            ind_acc = accp.tile([128, M_SLOTS], F32)
            cmax_acc = accp.tile([128, M_SLOTS], F32)
            loss_cols = accp.tile([128, CL // 128], F32)
            aux2_acc = accp.tile([128, 3 * CL // 128], F32)
            nc.vector.memset(ind_acc[:, :], 0.0)
            nc.vector.memset(cmax_acc[:, :], -3.0e38)

            for grp in range(N_GROUPS):
                g0 = grp * GROUP

                # ---------- loads ----------
                ttg = bigin.tile([128, 8, GROUP], BF16, tag="ttg")
                nc.sync.dma_start(ttg[:, :, :],
                                  tT.rearrange("(c p) t -> p c t", p=128)[:, :, g0:g0 + GROUP])
                itg = bigin.tile([128, 8, GROUP], F32R, tag="itg")
                nc.sync.dma_start(itg[:, :, :],
                                  iT.rearrange("(c p) t -> p c t", p=128)[:, :, g0:g0 + GROUP])

                # ---------- read path: softmax over slots ----------
                scT = ps_small.tile([M_SLOTS, GROUP], F32, tag="psr")
                for c in range(8):
                    nc.tensor.matmul(scT[:, :], ct_bf[:, c, :], ttg[:, c, :],
                                     start=(c == 0), stop=(c == 7))
                escT = small.tile([M_SLOTS, GROUP], F32R, tag="escT")
                nc.scalar.activation(escT[:, :], scT[:, :], ACTF.Exp)
                s_sum = ps_small.tile([1, GROUP], F32, tag="psr")
                nc.tensor.matmul(s_sum[:, :], (ones_sb[:M_SLOTS, :1]), (escT[:, :]),
                                 start=True, stop=True)
                ln_s = small.tile([1, GROUP], F32, tag="lns")
                nc.scalar.activation(ln_s[:, :], s_sum[:, :], ACTF.Ln)
                r_sum = small.tile([1, GROUP], F32R, tag="rsum")
                nc.scalar.activation(r_sum[:, :], ln_s[:, :], ACTF.Exp, scale=-1.0)
                bcast = ps_small.tile([M_SLOTS, GROUP], F32, tag="psr")
                nc.tensor.matmul(bcast[:, :], (ones_sb[:1, :M_SLOTS]), (r_sum[:, :]),
                                 start=True, stop=True)
                smT = small.tile([M_SLOTS, GROUP], BF16, tag="smT")
                nc.vector.tensor_tensor(smT[:, :], escT[:, :], bcast[:, :], ALU.mult)

                # ---------- write path: raw scores ----------
                srT = ps_small.tile([M_SLOTS, GROUP], F32, tag="pss")
                for c in range(8):
                    nc.tensor.matmul(srT[:, :], ct_sb[:, c, :], itg[:, c, :],
                                     start=(c == 0), stop=(c == 7))
                sr_sb = small.tile([M_SLOTS, GROUP], F32, tag="sr_sb")
                nc.scalar.copy(sr_sb[:, :], srT[:, :])

                rin_g = stat.tile([128, SUBS], F32, tag="rin_g")
                nc.sync.dma_start(
                    rin_g[:, :],
                    rin_in.rearrange("(g s p) o -> p (g s o)", p=128, s=SUBS)[:, grp * SUBS:(grp + 1) * SUBS])
                imt_h = {}
                t_h = {}
                tf_h = {}
                for hh in range(2):
                    imt_hh = rows.tile([128, 2, D], F32R, tag="imt")
                    imt_h[hh] = imt_hh
                    nc.sync.dma_start(
                        imt_h[hh][:, :, :],
                        im_r.rearrange("(q p) d -> p q d", p=128)[:, grp * SUBS + hh * 2: grp * SUBS + hh * 2 + 2, :])
                    t_hh = rows.tile([128, 2, D], F32, tag="t_sb")
                    t_h[hh] = t_hh
                    nc.sync.dma_start(
                        t_h[hh][:, :, :],
                        t_r.rearrange("(q p) d -> p q d", p=128)[:, grp * SUBS + hh * 2: grp * SUBS + hh * 2 + 2, :])
                    tf_hh = rows.tile([128, 2, D], F32, tag="tf_sb")
                    tf_h[hh] = tf_hh

                ss2g = stat.tile([128, SUBS], F32, tag="ss2g")
                rn2g = stat.tile([128, SUBS], F32, tag="rn2g")

                for s in range(SUBS):
                    tok0 = g0 + s * 128
                    tsl = slice(s * 128, (s + 1) * 128)
                    tile_idx = tok0 // 128

                    # ---- normalized image: rin precomputed on host ----
                    imt = imt_h[s // 2][:, s % 2, :]
                    rin = rin_g[:, s:s + 1]

                    # ---- s tile token-major ----
                    ps_t = ps_small.tile([128, M_SLOTS], F32, tag="pss")
                    nc.tensor.transpose(ps_t[:, :], sr_sb[:, tsl], ident[:M_SLOTS, :M_SLOTS])
                    s_tok = small.tile([128, M_SLOTS], F32, tag="s_tok")
                    nc.vector.tensor_scalar_mul(s_tok[:, :], ps_t[:, :], rin[:, :])

                    # ---- top-1 mask & weights ----
                    rmax = aux2_acc[:, 3 * tile_idx:3 * tile_idx + 1]
                    nc.vector.tensor_reduce(rmax, s_tok[:, :], AX.X, ALU.max)
                    e_tok = small.tile([128, M_SLOTS], F32, tag="e_tok")
                    nc.scalar.activation(e_tok[:, :], s_tok[:, :], ACTF.Exp)
                    ind = small.tile([128, M_SLOTS], F32, tag="ind")
                    nc.vector.tensor_scalar(ind[:, :], s_tok[:, :], rmax, None,
                                            op0=ALU.is_ge)
                    # top-2 value and argmax index (for host-side near-tie fixup)
                    msk = small.tile([128, M_SLOTS], F32, tag="msk")
                    nc.vector.scalar_tensor_tensor(msk[:, :], ind[:, :], -1.0e30,
                                                   s_tok[:, :], ALU.mult, ALU.add)
                    nc.vector.tensor_reduce(aux2_acc[:, 3 * tile_idx + 1:3 * tile_idx + 2],
                                            msk[:, :], AX.X, ALU.max)
                    nc.vector.tensor_tensor(msk[:, :], ind[:, :], iota_sb[:, :], ALU.mult)
                    nc.vector.tensor_reduce(aux2_acc[:, 3 * tile_idx + 2:3 * tile_idx + 3],
                                            msk[:, :], AX.X, ALU.max)
                    er = small.tile([128, M_SLOTS], F32, tag="er")
                    nc.vector.tensor_scalar_mul(er[:, :], e_tok[:, :], rin[:, :])
                    wm = small.tile([128, M_SLOTS], F32R, tag="wm")
                    nc.vector.tensor_tensor(wm[:, :], er[:, :], ind[:, :], ALU.mult)
                    nc.vector.tensor_tensor(cmax_acc[:, :], cmax_acc[:, :], s_tok[:, :], ALU.max)
                    nc.vector.tensor_tensor(ind_acc[:, :], ind_acc[:, :], ind[:, :], ALU.add)

                    # ---- slot sums (segment-sum as matmul) ----
                    first = (tile_idx == 0)
                    last = (tile_idx == CL // 128 - 1)
                    for h in range(2):
                        nc.tensor.matmul(u_psum[:, h * 512:(h + 1) * 512], (wm[:, :]),
                                         (imt[:, h * 512:(h + 1) * 512]),
                                         start=first, stop=last)

                    # ---- extractor ----
                    t_sb = t_h[s // 2][:, s % 2, :]
                    tf_sb = tf_h[s // 2][:, s % 2, :]
                    for h in range(2):
                        hsl = slice(h * 512, (h + 1) * 512)
                        tfp = ps_big.tile([128, 512], F32, tag="tfp")
                        for c in range(8):
                            nc.tensor.matmul(tfp[:, :], ttg[:, c, tsl], w1t_sb[:, c, hsl],
                                             start=(c == 0), stop=False)
                        nc.tensor.matmul(tfp[:, :], smT[:, tsl], g_sb[:, hsl],
                                         start=False, stop=True)
                        nc.vector.tensor_tensor(tf_sb[:, hsl], tfp[:, :], t_sb[:, hsl], ALU.add)
                    if s % 2 == 1:
                        nc.gpsimd.dma_start(
                            tf_out.rearrange("(q p) d -> p q d", p=128)[:, grp * SUBS + (s // 2) * 2: grp * SUBS + (s // 2) * 2 + 2, :],
                            tf_h[s // 2][:, :, :])

                    # ---- loss: row sum of tf^2 into the group stat tile ----
                    sq_scr = work.tile([128, D], F32, tag="scr")
                    nc.scalar.activation(sq_scr[:, :], tf_sb[:, :], ACTF.Square,
                                         accum_out=ss2g[:, s:s + 1])

                # ---- batched rn2 = rsqrt(ss2g): bit-hack seed + 3 Newton steps
                # (DVE-only, keeps ACT inside one activation-table set) ----
                sh = stat.tile([128, SUBS], mybir.dt.int32, tag="sh")
                nc.vector.tensor_scalar(sh[:, :], ss2g[:, :].bitcast(mybir.dt.int32),
                                        1, None, op0=ALU.logical_shift_right)
                nc.vector.scalar_tensor_tensor(rn2g[:, :].bitcast(mybir.dt.int32),
                                               sh[:, :], -1, magicb[:, :],
                                               ALU.mult, ALU.add)
                half = stat.tile([128, SUBS], F32, tag="half")
                nc.vector.tensor_scalar(half[:, :], ss2g[:, :], 0.5, None, op0=ALU.mult)
                t1 = stat.tile([128, SUBS], F32, tag="t1")
                for _ in range(3):
                    nc.vector.tensor_tensor(t1[:, :], rn2g[:, :], rn2g[:, :], ALU.mult)
                    nc.vector.tensor_tensor(t1[:, :], t1[:, :], half[:, :], ALU.mult)
                    nc.vector.tensor_scalar(t1[:, :], t1[:, :], 1.5, -1.0,
                                            op0=ALU.subtract, op1=ALU.mult)
                    nc.vector.tensor_tensor(rn2g[:, :], rn2g[:, :], t1[:, :], ALU.mult)

                for s in range(SUBS):
                    tile_idx = (grp * GROUP + s * 128) // 128
                    t_sb = t_h[s // 2][:, s % 2, :]
                    tf_sb = tf_h[s // 2][:, s % 2, :]
                    lntf = work.tile([128, D], F32, tag="lntf")
                    nc.vector.scalar_tensor_tensor(lntf[:, :], tf_sb[:, :],
                                                   rn2g[:, s:s + 1], t_sb[:, :],
                                                   ALU.mult, ALU.subtract)
                    abs_scr = work.tile([128, D], F32, tag="scr")
                    nc.scalar.activation(abs_scr[:, :], lntf[:, :], ACTF.Abs,
                                         accum_out=loss_cols[:, tile_idx:tile_idx + 1])

            # ---- finalize ----
            u_sb = wpool.tile([M_SLOTS, D], F32)
            nc.scalar.copy(u_sb[:, :], u_psum[:, :])
            nc.gpsimd.dma_start(u_out[:, :], u_sb[:, :])
            aux_sb = accp.tile([128, 51], F32)
            nc.vector.tensor_copy(aux_sb[:, 0:M_SLOTS], ind_acc[:, :])
            nc.vector.tensor_copy(aux_sb[:, M_SLOTS:2 * M_SLOTS], cmax_acc[:, :])
            nc.vector.tensor_reduce(aux_sb[:, 50:51], loss_cols[:, :], AX.X, ALU.add)
            nc.gpsimd.dma_start(aux_out[:, :], aux_sb[:, :])
            nc.gpsimd.dma_start(aux2_out[:, :], aux2_acc[:, :])

    nc.compile()
    return nc


def _l2norm_np(x, axis=-1, eps=1e-12):
    n = np.linalg.norm(x, axis=axis, keepdims=True)
    return x / np.maximum(n, eps)


def make_in_maps(text_token, image_token, cache, W_ext):
    text_token = np.asarray(text_token, dtype=np.float32)
    image_token = np.asarray(image_token, dtype=np.float32)
    cache = np.asarray(cache, dtype=np.float32)
    W_ext = np.asarray(W_ext, dtype=np.float32)

    import ml_dtypes
    bf16 = ml_dtypes.bfloat16
    tT = np.ascontiguousarray(text_token.T.astype(bf16))
    iT = np.ascontiguousarray(image_token.T)
    w1t = np.ascontiguousarray((ALPHA * W_ext[:, :D]).T.astype(bf16))
    g = np.ascontiguousarray((ALPHA * (cache @ W_ext[:, D:].T)).astype(bf16))
    ct = np.ascontiguousarray(cache.T)
    ct_bf = ct.astype(bf16)
    ident = np.eye(128, dtype=np.float32)
    nrm = np.linalg.norm(image_token.astype(np.float64), axis=1, keepdims=True)
    rin_full = (1.0 / np.maximum(nrm, 1e-12)).astype(np.float32)

    in_maps = []
    for c in range(N_CORES):
        sl = slice(c * CL, (c + 1) * CL)
        in_maps.append({
            "tT": np.ascontiguousarray(tT[:, sl]),
            "t_r": np.ascontiguousarray(text_token[sl]),
            "iT": np.ascontiguousarray(iT[:, sl]),
            "im_r": np.ascontiguousarray(image_token[sl]),
            "w1t": w1t,
            "g_in": g,
            "ct_in": ct,
            "ct_bf_in": ct_bf,
            "ident_in": ident,
            "ones_in": np.ones((128, 32), dtype=np.float32),
            "iota_in": np.tile(np.arange(M_SLOTS, dtype=np.float32), (128, 1)),
            "rin_in": rin_full[sl],
        })
    return in_maps


GAP_THR = 1.0e-3


def combine_outputs(results, cache, image_token):
    """results: list of 8 dicts with tf_out/u_out/aux_out/aux2_out. Host unshard.

    Near-tie argmax fixup: the device's s is fp32-accurate but summation order
    differs from the reference; for tokens whose top-2 gap < GAP_THR, recompute
    the slot scores in float64 and move the contribution if the device picked a
    different slot than the (essentially exact) fp64 argmax.
    """
    cache = np.asarray(cache, dtype=np.float32)
    tf = np.concatenate([r["tf_out"] for r in results], axis=0)

    u_sum = np.sum([r["u_out"] for r in results], axis=0).astype(np.float64)  # [25, 1024]
    ind_sum = np.sum([r["aux_out"][:, :M_SLOTS] for r in results], axis=(0, 1)).astype(np.float64)
    col_max = np.max([r["aux_out"][:, M_SLOTS:2 * M_SLOTS] for r in results], axis=(0, 1))
    loss = np.sum([r["aux_out"][:, 50] for r in results]) / (C_FULL * D)

    cache64 = cache.astype(np.float64)
    image_token = np.asarray(image_token)
    for c, r in enumerate(results):
        a2 = r["aux2_out"].astype(np.float64)          # [128, 3*ntiles]
        top1 = a2[:, 0::3]                              # [128, 32]
        top2 = a2[:, 1::3]
        idx = a2[:, 2::3]
        amb = np.argwhere(top1 - top2 < GAP_THR)
        for p, t in amb:
            gi = c * CL + t * 128 + p
            imr = image_token[gi].astype(np.float64)
            bf = imr / max(np.linalg.norm(imr), 1e-12)
            srow = bf @ cache64.T
            j_true = int(np.argmax(srow))
            j_dev = int(round(idx[p, t]))
            if j_dev != j_true:
                u_sum[j_dev] -= np.exp(srow[j_dev]) * bf
                u_sum[j_true] += np.exp(srow[j_true]) * bf
                ind_sum[j_dev] -= 1.0
                ind_sum[j_true] += 1.0

    slot_sum = u_sum * np.exp(-col_max)[:, None]
    counts = ind_sum
    updated = np.where(counts[:, None] > 0,
                       MOMENTUM * cache + (1.0 - MOMENTUM) * slot_sum,
                       cache)
    new_cache = _l2norm_np(updated).astype(np.float32)
    return tf.astype(np.float32), np.float32(loss), new_cache


_NC_CACHE = {}


def kernel(text_token, image_token, cache, W_ext):
    if "nc" not in _NC_CACHE:
        _NC_CACHE["nc"] = build_nc()
    nc = _NC_CACHE["nc"]
    in_maps = make_in_maps(text_token, image_token, cache, W_ext)
    res = bass_utils.run_bass_kernel_spmd(nc, in_maps, core_ids=list(range(N_CORES)))
    return combine_outputs(res.results, cache, image_token)
